# revision 8
# baseline (speedup 1.0000x reference)
"""Multi-head attention (B=16, N=1024, D=1024, H=8, dh=128) on 8 trn2 cores.

Strategy: data-parallel over batch (2 batches/core), fp32r matmuls.
Per batch on each core:
  phase 1 (per 2-head group g): Q^T_g, K^T_g (head-transposed: dh on
    partitions) and V_g (natural) via fp32r matmuls from x^T (host-side
    pre-transposed) and streamed weight slices.
  phase 2 (per head, per 512-wide q chunk): S^T = K_h^T.T @ Q_h^T (k on
    partitions), E^T = exp(norm*S^T) on ACT, heads^T += V_h.T @ E^T, and
    R = colsum(E^T) via DVE/GpSimd adds + ones-vector matmul; 1/R
    broadcast to all partitions with a K=1 outer-product matmul.
  phase 3: out = (heads^T).T @ Wo in natural layout, scaled by 1/R via
    the broadcast tile, plus bv@Wo row via a K=1 matmul.
"""

import numpy as np

import concourse.bass as bass
import concourse.mybir as mybir
import concourse.tile as tile
from concourse import bacc
from concourse.bass_utils import run_bass_kernel_spmd

N_CORES = 8
B = 16
BPC = B // N_CORES      # batches per core
N = 1024                # sequence length
D = 1024                # model dim
H = 8                   # heads
DH = 128                # head dim
P = 128
DB = D // P             # 8 contraction blocks
GH = 2                  # heads per group
G = H // GH             # 4 groups
GW = GH * DH            # 256: e-width per group
NC2 = N // 512          # 2 n-chunks of 512
NORM = 1.0 / np.sqrt(DH)

F32 = mybir.dt.float32
F32R = mybir.dt.float32r


def r(ap):
    return ap


def build_nc():
    nc = bacc.Bacc()
    xT = nc.declare_dram_parameter("xT", [BPC, D, N], F32R, isOutput=False)
    Wq = nc.declare_dram_parameter("Wq", [D, D], F32R, isOutput=False)
    Wk = nc.declare_dram_parameter("Wk", [D, D], F32R, isOutput=False)
    Wv = nc.declare_dram_parameter("Wv", [D, D], F32R, isOutput=False)
    Wo = nc.declare_dram_parameter("Wo", [D, D], F32R, isOutput=False)
    bq = nc.declare_dram_parameter("bq", [D], F32, isOutput=False)
    bk = nc.declare_dram_parameter("bk", [D], F32, isOutput=False)
    bv = nc.declare_dram_parameter("bv", [D], F32R, isOutput=False)
    out = nc.declare_dram_parameter("out", [BPC, N, D], F32, isOutput=True)

    ws = [Wq, Wk, Wv]

    with tile.TileContext(nc) as tc:
        with tc.tile_pool(name="big", bufs=1) as big, \
             tc.tile_pool(name="wp", bufs=1) as wp, \
             tc.tile_pool(name="work", bufs=1) as work, \
             tc.tile_pool(name="small", bufs=1) as small, \
             tc.tile_pool(name="ps", bufs=1, space="PSUM") as ps:

            # constants / biases
            bq_col = small.tile([P, DB], F32, name="bq_col")
            bk_col = small.tile([P, DB], F32, name="bk_col")
            bv_col = small.tile([P, DB], F32R, name="bv_col")
            nc.sync.dma_start(out=bq_col, in_=bq.rearrange("(eb p) -> p eb", p=P))
            nc.sync.dma_start(out=bk_col, in_=bk.rearrange("(eb p) -> p eb", p=P))
            nc.sync.dma_start(out=bv_col, in_=bv.rearrange("(eb p) -> p eb", p=P))
            ones_f32 = small.tile([P, 1], F32, name="ones_f32")
            nc.vector.memset(ones_f32, 1.0)
            ones_row_f32 = small.tile([1, P], F32, name="ones_row_f32")
            nc.vector.memset(ones_row_f32, 1.0)
            ones_col = small.tile([P, 1], F32R, name="ones_col")
            nc.vector.tensor_copy(ones_col, ones_f32)
            ones_row = small.tile([1, P], F32R, name="ones_row")
            nc.vector.tensor_copy(ones_row, ones_row_f32)
            c_sb = small.tile([1, NC2, 512], F32R, name="c_sb")

            for b in range(BPC):
                sfx = f"_b{b}"
                # ---- load x^T for this batch: [128, db, n]
                xt = big.tile([P, DB, N], F32R, name=f"xt{sfx}", tag="xt")
                nc.sync.dma_start(out=xt, in_=xT[b].rearrange("(db p) n -> p db n", p=P))

                hT = big.tile([P, H, N], F32R, name=f"hT{sfx}", tag="hT")

                for g in range(G):
                    gsfx = f"{sfx}_g{g}"
                    e0 = g * GW
                    # ---- weight slices for this group: [128, db, GW]
                    wgt = {}
                    for wi, wname in enumerate(("wq", "wk", "wv")):
                        wt = wp.tile([P, DB, GW], F32R, name=f"{wname}{gsfx}", tag="wg", bufs=3)
                        src = ws[wi].rearrange("(db p) e -> p db e", p=P)
                        nc.sync.dma_start(out=wt, in_=src[:, :, e0:e0 + GW])
                        wgt[wname] = wt

                    # ---- phase 1: projections for this group
                    qTg = work.tile([P, GH, N], F32R, name=f"qT{gsfx}", tag="qTg", bufs=2)
                    kTg = work.tile([P, GH, N], F32R, name=f"kT{gsfx}", tag="kTg", bufs=2)
                    vg = work.tile([P, DB, GW], F32R, name=f"v{gsfx}", tag="vg", bufs=2)

                    # Q^T, K^T: out [e(128) x n(512)], lhsT = W slice, rhs = x^T
                    for dst, wt, bcol in ((qTg, wgt["wq"], bq_col),
                                          (kTg, wgt["wk"], bk_col)):
                        for eb in range(GH):
                            for nch in range(NC2):
                                acc = ps.tile([P, 512], F32, tag="p512", bufs=4,
                                              name=f"pqk{gsfx}_{eb}_{nch}")
                                for db in range(DB):
                                    nc.tensor.matmul(
                                        acc,
                                        r(wt[:, db, eb * P:(eb + 1) * P]),
                                        r(xt[:, db, nch * 512:(nch + 1) * 512]),
                                        start=(db == 0), stop=(db == DB - 1))
                                ebg = (e0 // P) + eb
                                nc.vector.tensor_scalar_add(
                                    dst[:, eb, nch * 512:(nch + 1) * 512],
                                    acc, bcol[:, ebg:ebg + 1])

                    # V natural: out [n(128) x e(GW=256)], lhsT = x^T, rhs = Wv
                    for nb in range(DB):
                        accv = ps.tile([P, 512], F32, tag="p512", bufs=4,
                                       name=f"pv{gsfx}_{nb}")
                        for db in range(DB):
                            nc.tensor.matmul(
                                accv[:, :GW],
                                r(xt[:, db, nb * P:(nb + 1) * P]),
                                r(wgt["wv"][:, db, :]),
                                start=(db == 0), stop=(db == DB - 1))
                        nc.vector.tensor_copy(vg[:, nb, :], accv[:, :GW])

                    # ---- phase 2: attention for the heads of this group
                    for hh in range(GH):
                        h = g * GH + hh
                        for qc in range(NC2):
                            asfx = f"{sfx}_h{h}_q{qc}"
                            eT = work.tile([P, 4, 1024], F32R, name=f"eT{asfx}",
                                           tag="eT", bufs=2)
                            for j in range(4):
                                # scores for kb=2j, 2j+1 into one 2-bank tile
                                sp = ps.tile([P, 1024], F32, tag="spair", bufs=2,
                                             name=f"sp{asfx}_{j}")
                                for half in range(2):
                                    kb = 2 * j + half
                                    nc.tensor.matmul(
                                        sp[:, half * 512:(half + 1) * 512],
                                        r(kTg[:, hh, kb * P:(kb + 1) * P]),
                                        r(qTg[:, hh, qc * 512:(qc + 1) * 512]),
                                        start=True, stop=True)
                                nc.scalar.activation(
                                    eT[:, j, :], sp,
                                    mybir.ActivationFunctionType.Exp,
                                    scale=float(NORM))

                            # heads^T (unnormalized): [dv(128) x q(512)]
                            pav = ps.tile([P, 512], F32, tag="p512", bufs=4,
                                          name=f"pav{asfx}")
                            for j in range(4):
                                for half in range(2):
                                    kb = 2 * j + half
                                    nc.tensor.matmul(
                                        pav,
                                        r(vg[:, kb, hh * DH:(hh + 1) * DH]),
                                        r(eT[:, j, half * 512:(half + 1) * 512]),
                                        start=(kb == 0), stop=(kb == DB - 1))

                            # R = col-sum of E^T: pairwise adds split DVE/GpSimd
                            t0 = work.tile([P, 512], F32R, name=f"t0{asfx}", tag="t0")
                            t1 = work.tile([P, 512], F32R, name=f"t1{asfx}", tag="t1")
                            t2 = work.tile([P, 512], F32R, name=f"t2{asfx}", tag="t2")
                            t3 = work.tile([P, 512], F32R, name=f"t3{asfx}", tag="t3")
                            add = mybir.AluOpType.add
                            nc.vector.tensor_tensor(t0, eT[:, 0, 0:512], eT[:, 0, 512:1024], add)
                            nc.gpsimd.tensor_tensor(t1, eT[:, 1, 0:512], eT[:, 1, 512:1024], add)
                            nc.vector.tensor_tensor(t2, eT[:, 2, 0:512], eT[:, 2, 512:1024], add)
                            nc.gpsimd.tensor_tensor(t3, eT[:, 3, 0:512], eT[:, 3, 512:1024], add)
                            nc.vector.tensor_tensor(t0, t0, t1, add)
                            nc.vector.tensor_tensor(t2, t2, t3, add)
                            nc.vector.tensor_tensor(t0, t0, t2, add)
                            pr = ps.tile([1, 512], F32, tag="p512", bufs=4, name=f"pr{asfx}")
                            nc.tensor.matmul(pr, r(ones_col), r(t0),
                                             start=True, stop=True)
                            rinv = small.tile([1, 512], F32R, name=f"rinv{asfx}",
                                              tag="rinv", bufs=1)
                            with nc.allow_low_precision(reason="softmax 1/R rounded to fp32r"):
                                nc.vector.reciprocal(rinv, pr)
                            # broadcast 1/R to 128 partitions via K=1 matmul
                            pbc = ps.tile([P, 512], F32, tag="p512", bufs=4, name=f"pbc{asfx}")
                            nc.tensor.matmul(pbc, r(ones_row), r(rinv),
                                             start=True, stop=True)
                            bc = work.tile([P, 512], F32, name=f"bc{asfx}",
                                           tag="bc", bufs=1)
                            nc.vector.tensor_copy(bc, pbc)
                            nc.vector.tensor_tensor(
                                hT[:, h, qc * 512:(qc + 1) * 512], pav, bc,
                                mybir.AluOpType.mult)

                # ---- phase 3: out = heads_norm @ Wo + bv@Wo (natural layout)
                for oc in range(NC2):
                    wo = wp.tile([P, DB, 512], F32R, name=f"wo{sfx}_{oc}", tag="wo")
                    src = Wo.rearrange("(eb p) o -> p eb o", p=P)
                    nc.sync.dma_start(out=wo, in_=src[:, :, oc * 512:(oc + 1) * 512])
                    if b == 0:
                        # c = bv @ Wo for this o-chunk (once; reused for b=1)
                        pc = ps.tile([1, 512], F32, tag="p512", bufs=4, name=f"pc_{oc}")
                        for eb in range(DB):
                            nc.tensor.matmul(pc, r(bv_col[:, eb:eb + 1]),
                                             r(wo[:, eb, :]),
                                             start=(eb == 0), stop=(eb == DB - 1))
                        nc.vector.tensor_copy(c_sb[:, oc, :], pc)
                    for nb in range(DB):
                        po = ps.tile([P, 512], F32, tag="p512", bufs=4,
                                     name=f"po{sfx}_{oc}_{nb}")
                        for eb in range(H):
                            nc.tensor.matmul(
                                po,
                                r(hT[:, eb, nb * P:(nb + 1) * P]),
                                r(wo[:, eb, :]),
                                start=(eb == 0), stop=False)
                        nc.tensor.matmul(po, r(ones_row), r(c_sb[:, oc, :]),
                                         start=False, stop=True)
                        osb = work.tile([P, 512], F32, name=f"o{sfx}_{oc}_{nb}",
                                        tag="osb", bufs=2)
                        nc.scalar.activation(osb, po,
                                             mybir.ActivationFunctionType.Copy)
                        nc.sync.dma_start(
                            out=out[b, nb * P:(nb + 1) * P, oc * 512:(oc + 1) * 512],
                            in_=osb)

    nc.compile()
    return nc


_NC_CACHE = None


def _get_nc():
    global _NC_CACHE
    if _NC_CACHE is None:
        _NC_CACHE = build_nc()
    return _NC_CACHE


def make_in_maps(x, Wq, bq, Wk, bk, Wv, bv, Wo):
    x = np.asarray(x, dtype=np.float32)
    in_maps = []
    shared = {
        "Wq": np.ascontiguousarray(Wq, dtype=np.float32),
        "Wk": np.ascontiguousarray(Wk, dtype=np.float32),
        "Wv": np.ascontiguousarray(Wv, dtype=np.float32),
        "Wo": np.ascontiguousarray(Wo, dtype=np.float32),
        "bq": np.ascontiguousarray(bq, dtype=np.float32),
        "bk": np.ascontiguousarray(bk, dtype=np.float32),
        "bv": np.ascontiguousarray(bv, dtype=np.float32),
    }
    for c in range(N_CORES):
        xc = x[c * BPC:(c + 1) * BPC]                 # [BPC, N, D]
        xTc = np.ascontiguousarray(xc.transpose(0, 2, 1))  # [BPC, D, N]
        in_maps.append({"xT": xTc, **shared})
    return in_maps


def run(x, Wq, bq, Wk, bk, Wv, bv, Wo, trace=False):
    nc = _get_nc()
    in_maps = make_in_maps(x, Wq, bq, Wk, bk, Wv, bv, Wo)
    res = run_bass_kernel_spmd(nc, in_maps, list(range(N_CORES)), trace=trace)
    out = np.concatenate([res.results[c]["out"] for c in range(N_CORES)], axis=0)
    return out, res


def kernel(x, Wq, bq, Wk, bk, Wv, bv, Wo):
    out, _ = run(x, Wq, bq, Wk, bk, Wv, bv, Wo, trace=False)
    return out


# revision 10
# speedup vs baseline: 1.2304x; 1.2304x over previous
"""Multi-head attention (B=16, N=1024, D=1024, H=8, dh=128) on 8 trn2 cores.

Strategy: data-parallel over batch (2 batches/core), fp32r matmuls.
Per batch on each core:
  phase 1 (per 2-head group g): Q^T_g, K^T_g (head-transposed: dh on
    partitions) and V_g (natural) via fp32r matmuls from x^T (host-side
    pre-transposed) and streamed weight slices.
  phase 2 (per head, per 512-wide q chunk): S^T = K_h^T.T @ Q_h^T (k on
    partitions), E^T = exp(norm*S^T) on ACT, heads^T += V_h.T @ E^T, and
    R = colsum(E^T) via DVE/GpSimd adds + ones-vector matmul; 1/R
    broadcast to all partitions with a K=1 outer-product matmul.
  phase 3: out = (heads^T).T @ Wo in natural layout, scaled by 1/R via
    the broadcast tile, plus bv@Wo row via a K=1 matmul.
"""

import numpy as np

import concourse.bass as bass
import concourse.mybir as mybir
import concourse.tile as tile
from concourse import bacc
from concourse.bass_utils import run_bass_kernel_spmd

N_CORES = 8
B = 16
BPC = B // N_CORES      # batches per core
N = 1024                # sequence length
D = 1024                # model dim
H = 8                   # heads
DH = 128                # head dim
P = 128
DB = D // P             # 8 contraction blocks
GH = 2                  # heads per group
G = H // GH             # 4 groups
GW = GH * DH            # 256: e-width per group
NC2 = N // 512          # 2 n-chunks of 512
NORM = 1.0 / np.sqrt(DH)

F32 = mybir.dt.float32
F32R = mybir.dt.float32r


def r(ap):
    return ap


def build_nc():
    nc = bacc.Bacc()
    xT = nc.declare_dram_parameter("xT", [BPC, D, N], F32R, isOutput=False)
    Wq = nc.declare_dram_parameter("Wq", [D, D], F32R, isOutput=False)
    Wk = nc.declare_dram_parameter("Wk", [D, D], F32R, isOutput=False)
    Wv = nc.declare_dram_parameter("Wv", [D, D], F32R, isOutput=False)
    Wo = nc.declare_dram_parameter("Wo", [D, D], F32R, isOutput=False)
    bq = nc.declare_dram_parameter("bq", [D], F32, isOutput=False)
    bk = nc.declare_dram_parameter("bk", [D], F32, isOutput=False)
    bv = nc.declare_dram_parameter("bv", [D], F32R, isOutput=False)
    out = nc.declare_dram_parameter("out", [BPC, N, D], F32, isOutput=True)

    ws = [Wq, Wk, Wv]

    with tile.TileContext(nc) as tc:
        with tc.tile_pool(name="big", bufs=1) as big, \
             tc.tile_pool(name="wp", bufs=1) as wp, \
             tc.tile_pool(name="work", bufs=1) as work, \
             tc.tile_pool(name="small", bufs=1) as small, \
             tc.tile_pool(name="ps", bufs=1, space="PSUM") as ps:

            # constants / biases
            bq_col = small.tile([P, DB], F32, name="bq_col")
            bk_col = small.tile([P, DB], F32, name="bk_col")
            bv_col = small.tile([P, DB], F32R, name="bv_col")
            nc.sync.dma_start(out=bq_col, in_=bq.rearrange("(eb p) -> p eb", p=P))
            nc.sync.dma_start(out=bk_col, in_=bk.rearrange("(eb p) -> p eb", p=P))
            nc.sync.dma_start(out=bv_col, in_=bv.rearrange("(eb p) -> p eb", p=P))
            ones_f32 = small.tile([P, 1], F32, name="ones_f32")
            nc.vector.memset(ones_f32, 1.0)
            ones_row_f32 = small.tile([1, P], F32, name="ones_row_f32")
            nc.vector.memset(ones_row_f32, 1.0)
            ones_col = small.tile([P, 1], F32R, name="ones_col")
            nc.vector.tensor_copy(ones_col, ones_f32)
            ones_row = small.tile([1, P], F32R, name="ones_row")
            nc.vector.tensor_copy(ones_row, ones_row_f32)
            c_sb = small.tile([1, NC2, 512], F32R, name="c_sb")

            for b in range(BPC):
                sfx = f"_b{b}"
                # ---- load x^T for this batch: [128, db, n]
                xt = big.tile([P, DB, N], F32R, name=f"xt{sfx}", tag="xt")
                nc.sync.dma_start(out=xt, in_=xT[b].rearrange("(db p) n -> p db n", p=P))

                hT = big.tile([P, H, N], F32R, name=f"hT{sfx}", tag="hT")

                for g in range(G):
                    gsfx = f"{sfx}_g{g}"
                    e0 = g * GW
                    # ---- weight slices for this group: [128, db, GW]
                    wgt = {}
                    for wi, wname in enumerate(("wq", "wk", "wv")):
                        wt = wp.tile([P, DB, GW], F32R, name=f"{wname}{gsfx}", tag="wg", bufs=2)
                        src = ws[wi].rearrange("(db p) e -> p db e", p=P)
                        nc.sync.dma_start(out=wt, in_=src[:, :, e0:e0 + GW])
                        wgt[wname] = wt

                    # ---- phase 1: projections for this group
                    qTg = work.tile([P, GH, N], F32R, name=f"qT{gsfx}", tag="qTg", bufs=2)
                    kTg = work.tile([P, GH, N], F32R, name=f"kT{gsfx}", tag="kTg", bufs=2)
                    vg = work.tile([P, DB, GW], F32R, name=f"v{gsfx}", tag="vg", bufs=2)

                    # Q^T, K^T: out [e(128) x n(512)], lhsT = W slice, rhs = x^T
                    for dst, wt, bcol in ((qTg, wgt["wq"], bq_col),
                                          (kTg, wgt["wk"], bk_col)):
                        for eb in range(GH):
                            for nch in range(NC2):
                                acc = ps.tile([P, 512], F32, tag="p512", bufs=4,
                                              name=f"pqk{gsfx}_{eb}_{nch}")
                                for db in range(DB):
                                    nc.tensor.matmul(
                                        acc,
                                        r(wt[:, db, eb * P:(eb + 1) * P]),
                                        r(xt[:, db, nch * 512:(nch + 1) * 512]),
                                        start=(db == 0), stop=(db == DB - 1))
                                ebg = (e0 // P) + eb
                                nc.vector.tensor_scalar_add(
                                    dst[:, eb, nch * 512:(nch + 1) * 512],
                                    acc, bcol[:, ebg:ebg + 1])

                    # V natural: out [n(128) x e(GW=256)], lhsT = x^T, rhs = Wv
                    for nb in range(DB):
                        accv = ps.tile([P, 512], F32, tag="p512", bufs=4,
                                       name=f"pv{gsfx}_{nb}")
                        for db in range(DB):
                            nc.tensor.matmul(
                                accv[:, :GW],
                                r(xt[:, db, nb * P:(nb + 1) * P]),
                                r(wgt["wv"][:, db, :]),
                                start=(db == 0), stop=(db == DB - 1))
                        nc.vector.tensor_copy(vg[:, nb, :], accv[:, :GW])

                    # ---- phase 2: attention for the heads of this group
                    for hh in range(GH):
                        h = g * GH + hh
                        for qc in range(NC2):
                            asfx = f"{sfx}_h{h}_q{qc}"
                            eT = work.tile([P, 4, 1024], F32R, name=f"eT{asfx}",
                                           tag="eT", bufs=2)
                            for j in range(4):
                                # scores for kb=2j, 2j+1 into one 2-bank tile
                                sp = ps.tile([P, 1024], F32, tag="spair", bufs=2,
                                             name=f"sp{asfx}_{j}")
                                for half in range(2):
                                    kb = 2 * j + half
                                    nc.tensor.matmul(
                                        sp[:, half * 512:(half + 1) * 512],
                                        r(kTg[:, hh, kb * P:(kb + 1) * P]),
                                        r(qTg[:, hh, qc * 512:(qc + 1) * 512]),
                                        start=True, stop=True)
                                nc.scalar.activation(
                                    eT[:, j, :], sp,
                                    mybir.ActivationFunctionType.Exp,
                                    scale=float(NORM))

                            # heads^T (unnormalized): [dv(128) x q(512)]
                            pav = ps.tile([P, 512], F32, tag="p512", bufs=4,
                                          name=f"pav{asfx}")
                            for j in range(4):
                                for half in range(2):
                                    kb = 2 * j + half
                                    nc.tensor.matmul(
                                        pav,
                                        r(vg[:, kb, hh * DH:(hh + 1) * DH]),
                                        r(eT[:, j, half * 512:(half + 1) * 512]),
                                        start=(kb == 0), stop=(kb == DB - 1))

                            # R = col-sum of E^T: pairwise adds split DVE/GpSimd
                            add = mybir.AluOpType.add
                            tA = work.tile([P, 1024], F32R, name=f"tA{asfx}", tag="tA", bufs=1)
                            tB = work.tile([P, 1024], F32R, name=f"tB{asfx}", tag="tB", bufs=1)
                            rp = work.tile([P, 512], F32R, name=f"rp{asfx}", tag="rp", bufs=2)
                            nc.vector.tensor_tensor(tA, eT[:, 0, :], eT[:, 1, :], add)
                            nc.vector.tensor_tensor(tB, eT[:, 2, :], eT[:, 3, :], add)
                            nc.vector.tensor_tensor(tA, tA, tB, add)
                            nc.vector.tensor_tensor(rp, tA[:, 0:512], tA[:, 512:1024], add)
                            pr = ps.tile([1, 512], F32, tag="p512", bufs=4, name=f"pr{asfx}")
                            nc.tensor.matmul(pr, r(ones_col), r(rp),
                                             start=True, stop=True)
                            # R row to SBUF (rounded to fp32r) then broadcast to
                            # all partitions via a K=1 outer-product matmul
                            r_row = small.tile([1, 512], F32R, name=f"rrow{asfx}",
                                               tag="rinv", bufs=1)
                            nc.vector.tensor_copy(r_row, pr)
                            pbc = ps.tile([P, 512], F32, tag="p512", bufs=4, name=f"pbc{asfx}")
                            nc.tensor.matmul(pbc, r(ones_row), r(r_row),
                                             start=True, stop=True)
                            # 1/R at full 128-lane width (approx + one NR pass)
                            scratch = work.tile([P, 512], F32, name=f"sc{asfx}",
                                                tag="bc", bufs=1)
                            binv = work.tile([P, 512], F32, name=f"binv{asfx}",
                                             tag="binv", bufs=1)
                            nc.vector.reciprocal_approx_accurate(binv, pbc, scratch)
                            nc.vector.tensor_tensor(
                                hT[:, h, qc * 512:(qc + 1) * 512], pav, binv,
                                mybir.AluOpType.mult)

                # ---- phase 3: out = heads_norm @ Wo + bv@Wo (natural layout)
                for oc in range(NC2):
                    wo = wp.tile([P, DB, 512], F32R, name=f"wo{sfx}_{oc}", tag="wo")
                    src = Wo.rearrange("(eb p) o -> p eb o", p=P)
                    nc.sync.dma_start(out=wo, in_=src[:, :, oc * 512:(oc + 1) * 512])
                    if b == 0:
                        # c = bv @ Wo for this o-chunk (once; reused for b=1)
                        pc = ps.tile([1, 512], F32, tag="p512", bufs=4, name=f"pc_{oc}")
                        for eb in range(DB):
                            nc.tensor.matmul(pc, r(bv_col[:, eb:eb + 1]),
                                             r(wo[:, eb, :]),
                                             start=(eb == 0), stop=(eb == DB - 1))
                        nc.vector.tensor_copy(c_sb[:, oc, :], pc)
                    for nb in range(DB):
                        po = ps.tile([P, 512], F32, tag="p512", bufs=4,
                                     name=f"po{sfx}_{oc}_{nb}")
                        for eb in range(H):
                            nc.tensor.matmul(
                                po,
                                r(hT[:, eb, nb * P:(nb + 1) * P]),
                                r(wo[:, eb, :]),
                                start=(eb == 0), stop=False)
                        nc.tensor.matmul(po, r(ones_row), r(c_sb[:, oc, :]),
                                         start=False, stop=True)
                        osb = work.tile([P, 512], F32, name=f"o{sfx}_{oc}_{nb}",
                                        tag="osb", bufs=2)
                        nc.scalar.activation(osb, po,
                                             mybir.ActivationFunctionType.Copy)
                        nc.sync.dma_start(
                            out=out[b, nb * P:(nb + 1) * P, oc * 512:(oc + 1) * 512],
                            in_=osb)

    nc.compile()
    return nc


_NC_CACHE = None


def _get_nc():
    global _NC_CACHE
    if _NC_CACHE is None:
        _NC_CACHE = build_nc()
    return _NC_CACHE


def make_in_maps(x, Wq, bq, Wk, bk, Wv, bv, Wo):
    x = np.asarray(x, dtype=np.float32)
    in_maps = []
    shared = {
        "Wq": np.ascontiguousarray(Wq, dtype=np.float32),
        "Wk": np.ascontiguousarray(Wk, dtype=np.float32),
        "Wv": np.ascontiguousarray(Wv, dtype=np.float32),
        "Wo": np.ascontiguousarray(Wo, dtype=np.float32),
        "bq": np.ascontiguousarray(bq, dtype=np.float32),
        "bk": np.ascontiguousarray(bk, dtype=np.float32),
        "bv": np.ascontiguousarray(bv, dtype=np.float32),
    }
    for c in range(N_CORES):
        xc = x[c * BPC:(c + 1) * BPC]                 # [BPC, N, D]
        xTc = np.ascontiguousarray(xc.transpose(0, 2, 1))  # [BPC, D, N]
        in_maps.append({"xT": xTc, **shared})
    return in_maps


def run(x, Wq, bq, Wk, bk, Wv, bv, Wo, trace=False):
    nc = _get_nc()
    in_maps = make_in_maps(x, Wq, bq, Wk, bk, Wv, bv, Wo)
    res = run_bass_kernel_spmd(nc, in_maps, list(range(N_CORES)), trace=trace)
    out = np.concatenate([res.results[c]["out"] for c in range(N_CORES)], axis=0)
    return out, res


def kernel(x, Wq, bq, Wk, bk, Wv, bv, Wo):
    out, _ = run(x, Wq, bq, Wk, bk, Wv, bv, Wo, trace=False)
    return out


# revision 11
# speedup vs baseline: 1.2354x; 1.0040x over previous
"""Multi-head attention (B=16, N=1024, D=1024, H=8, dh=128) on 8 trn2 cores.

Strategy: data-parallel over batch (2 batches/core), fp32r matmuls.
Per batch on each core:
  phase 1 (per 2-head group g): Q^T_g, K^T_g (head-transposed: dh on
    partitions) and V_g (natural) via fp32r matmuls from x^T (host-side
    pre-transposed) and streamed weight slices.
  phase 2 (per head, per 512-wide q chunk): S^T = K_h^T.T @ Q_h^T (k on
    partitions), E^T = exp(norm*S^T) on ACT, heads^T += V_h.T @ E^T, and
    R = colsum(E^T) via DVE/GpSimd adds + ones-vector matmul; 1/R
    broadcast to all partitions with a K=1 outer-product matmul.
  phase 3: out = (heads^T).T @ Wo in natural layout, scaled by 1/R via
    the broadcast tile, plus bv@Wo row via a K=1 matmul.
"""

import numpy as np

import concourse.bass as bass
import concourse.mybir as mybir
import concourse.tile as tile
from concourse import bacc
from concourse.bass_utils import run_bass_kernel_spmd

N_CORES = 8
B = 16
BPC = B // N_CORES      # batches per core
N = 1024                # sequence length
D = 1024                # model dim
H = 8                   # heads
DH = 128                # head dim
P = 128
DB = D // P             # 8 contraction blocks
GH = 2                  # heads per group
G = H // GH             # 4 groups
GW = GH * DH            # 256: e-width per group
NC2 = N // 512          # 2 n-chunks of 512
NORM = 1.0 / np.sqrt(DH)

F32 = mybir.dt.float32
F32R = mybir.dt.float32r


def r(ap):
    return ap


def build_nc():
    nc = bacc.Bacc()
    xT = nc.declare_dram_parameter("xT", [BPC, D, N], F32R, isOutput=False)
    Wq = nc.declare_dram_parameter("Wq", [D, D], F32R, isOutput=False)
    Wk = nc.declare_dram_parameter("Wk", [D, D], F32R, isOutput=False)
    Wv = nc.declare_dram_parameter("Wv", [D, D], F32R, isOutput=False)
    Wo = nc.declare_dram_parameter("Wo", [D, D], F32R, isOutput=False)
    bq = nc.declare_dram_parameter("bq", [D], F32, isOutput=False)
    bk = nc.declare_dram_parameter("bk", [D], F32, isOutput=False)
    bv = nc.declare_dram_parameter("bv", [D], F32R, isOutput=False)
    out = nc.declare_dram_parameter("out", [BPC, N, D], F32, isOutput=True)

    ws = [Wq, Wk, Wv]

    with tile.TileContext(nc) as tc:
        with tc.tile_pool(name="big", bufs=1) as big, \
             tc.tile_pool(name="wp", bufs=1) as wp, \
             tc.tile_pool(name="work", bufs=1) as work, \
             tc.tile_pool(name="small", bufs=1) as small, \
             tc.tile_pool(name="ps", bufs=1, space="PSUM") as ps:

            # constants / biases
            bq_col = small.tile([P, DB], F32, name="bq_col")
            bk_col = small.tile([P, DB], F32, name="bk_col")
            bv_col = small.tile([P, DB], F32R, name="bv_col")
            nc.sync.dma_start(out=bq_col, in_=bq.rearrange("(eb p) -> p eb", p=P))
            nc.sync.dma_start(out=bk_col, in_=bk.rearrange("(eb p) -> p eb", p=P))
            nc.sync.dma_start(out=bv_col, in_=bv.rearrange("(eb p) -> p eb", p=P))
            ones_f32 = small.tile([P, 1], F32, name="ones_f32")
            nc.vector.memset(ones_f32, 1.0)
            ones_row_f32 = small.tile([1, P], F32, name="ones_row_f32")
            nc.vector.memset(ones_row_f32, 1.0)
            ones_col = small.tile([P, 1], F32R, name="ones_col")
            nc.vector.tensor_copy(ones_col, ones_f32)
            ones_row = small.tile([1, P], F32R, name="ones_row")
            nc.vector.tensor_copy(ones_row, ones_row_f32)
            c_sb = small.tile([1, NC2, 512], F32R, name="c_sb")

            for b in range(BPC):
                sfx = f"_b{b}"
                # ---- load x^T for this batch: [128, db, n]
                xt = big.tile([P, DB, N], F32R, name=f"xt{sfx}", tag="xt")
                nc.sync.dma_start(out=xt, in_=xT[b].rearrange("(db p) n -> p db n", p=P))

                hT = big.tile([P, H, N], F32R, name=f"hT{sfx}", tag="hT")

                for g in range(G):
                    gsfx = f"{sfx}_g{g}"
                    e0 = g * GW
                    # ---- weight slices for this group: [128, db, GW]
                    wgt = {}
                    for wi, wname in enumerate(("wq", "wk", "wv")):
                        wt = wp.tile([P, DB, GW], F32R, name=f"{wname}{gsfx}", tag="wg", bufs=2)
                        src = ws[wi].rearrange("(db p) e -> p db e", p=P)
                        nc.sync.dma_start(out=wt, in_=src[:, :, e0:e0 + GW])
                        wgt[wname] = wt

                    # ---- phase 1: projections for this group
                    qTg = work.tile([P, GH, N], F32R, name=f"qT{gsfx}", tag="qTg", bufs=2)
                    kTg = work.tile([P, GH, N], F32R, name=f"kT{gsfx}", tag="kTg", bufs=2)
                    vg = work.tile([P, DB, GW], F32R, name=f"v{gsfx}", tag="vg", bufs=2)

                    # Q^T, K^T: out [e(128) x n(512)], lhsT = W slice, rhs = x^T
                    for dst, wt, bcol in ((qTg, wgt["wq"], bq_col),
                                          (kTg, wgt["wk"], bk_col)):
                        for eb in range(GH):
                            for nch in range(NC2):
                                acc = ps.tile([P, 512], F32, tag="pj", bufs=2,
                                              name=f"pqk{gsfx}_{eb}_{nch}")
                                for db in range(DB):
                                    nc.tensor.matmul(
                                        acc,
                                        r(wt[:, db, eb * P:(eb + 1) * P]),
                                        r(xt[:, db, nch * 512:(nch + 1) * 512]),
                                        start=(db == 0), stop=(db == DB - 1))
                                ebg = (e0 // P) + eb
                                nc.vector.tensor_scalar_add(
                                    dst[:, eb, nch * 512:(nch + 1) * 512],
                                    acc, bcol[:, ebg:ebg + 1])

                    # V natural: out [n(128) x e(GW=256)], lhsT = x^T, rhs = Wv
                    for nb in range(DB):
                        accv = ps.tile([P, 512], F32, tag="pj", bufs=2,
                                       name=f"pv{gsfx}_{nb}")
                        for db in range(DB):
                            nc.tensor.matmul(
                                accv[:, :GW],
                                r(xt[:, db, nb * P:(nb + 1) * P]),
                                r(wgt["wv"][:, db, :]),
                                start=(db == 0), stop=(db == DB - 1))
                        nc.vector.tensor_copy(vg[:, nb, :], accv[:, :GW])

                    # ---- phase 2: attention for the heads of this group
                    for hh in range(GH):
                        h = g * GH + hh
                        for qc in range(NC2):
                            asfx = f"{sfx}_h{h}_q{qc}"
                            eT = work.tile([P, 4, 1024], F32R, name=f"eT{asfx}",
                                           tag="eT", bufs=2)
                            for j in range(4):
                                # scores for kb=2j, 2j+1 into one 2-bank tile
                                sp = ps.tile([P, 1024], F32, tag="spair", bufs=2,
                                             name=f"sp{asfx}_{j}")
                                for half in range(2):
                                    kb = 2 * j + half
                                    nc.tensor.matmul(
                                        sp[:, half * 512:(half + 1) * 512],
                                        r(kTg[:, hh, kb * P:(kb + 1) * P]),
                                        r(qTg[:, hh, qc * 512:(qc + 1) * 512]),
                                        start=True, stop=True)
                                nc.scalar.activation(
                                    eT[:, j, :], sp,
                                    mybir.ActivationFunctionType.Exp,
                                    scale=float(NORM))

                            # heads^T (unnormalized): [dv(128) x q(512)]
                            pav = ps.tile([P, 512], F32, tag="pav", bufs=1,
                                          name=f"pav{asfx}")
                            for j in range(4):
                                for half in range(2):
                                    kb = 2 * j + half
                                    nc.tensor.matmul(
                                        pav,
                                        r(vg[:, kb, hh * DH:(hh + 1) * DH]),
                                        r(eT[:, j, half * 512:(half + 1) * 512]),
                                        start=(kb == 0), stop=(kb == DB - 1))

                            # R = col-sum of E^T: pairwise adds split DVE/GpSimd
                            add = mybir.AluOpType.add
                            tA = work.tile([P, 1024], F32R, name=f"tA{asfx}", tag="tA", bufs=1)
                            tB = work.tile([P, 1024], F32R, name=f"tB{asfx}", tag="tB", bufs=1)
                            rp = work.tile([P, 512], F32R, name=f"rp{asfx}", tag="rp", bufs=2)
                            nc.vector.tensor_tensor(tA, eT[:, 0, :], eT[:, 1, :], add)
                            nc.vector.tensor_tensor(tB, eT[:, 2, :], eT[:, 3, :], add)
                            nc.vector.tensor_tensor(tA, tA, tB, add)
                            nc.vector.tensor_tensor(rp, tA[:, 0:512], tA[:, 512:1024], add)
                            pr = ps.tile([1, 512], F32, tag="pnorm", bufs=1, name=f"pr{asfx}")
                            nc.tensor.matmul(pr, r(ones_col), r(rp),
                                             start=True, stop=True)
                            # R row to SBUF (rounded to fp32r) then broadcast to
                            # all partitions via a K=1 outer-product matmul
                            r_row = small.tile([1, 512], F32R, name=f"rrow{asfx}",
                                               tag="rinv", bufs=1)
                            nc.vector.tensor_copy(r_row, pr)
                            pbc = ps.tile([P, 512], F32, tag="pnorm", bufs=1, name=f"pbc{asfx}")
                            nc.tensor.matmul(pbc, r(ones_row), r(r_row),
                                             start=True, stop=True)
                            # 1/R at full 128-lane width (approx + one NR pass)
                            scratch = work.tile([P, 512], F32, name=f"sc{asfx}",
                                                tag="bc", bufs=1)
                            binv = work.tile([P, 512], F32, name=f"binv{asfx}",
                                             tag="binv", bufs=1)
                            nc.vector.reciprocal_approx_accurate(binv, pbc, scratch)
                            nc.vector.tensor_tensor(
                                hT[:, h, qc * 512:(qc + 1) * 512], pav, binv,
                                mybir.AluOpType.mult)

                # ---- phase 3: out = heads_norm @ Wo + bv@Wo (natural layout)
                for oc in range(NC2):
                    wo = wp.tile([P, DB, 512], F32R, name=f"wo{sfx}_{oc}", tag="wo")
                    src = Wo.rearrange("(eb p) o -> p eb o", p=P)
                    nc.sync.dma_start(out=wo, in_=src[:, :, oc * 512:(oc + 1) * 512])
                    if b == 0:
                        # c = bv @ Wo for this o-chunk (once; reused for b=1)
                        pc = ps.tile([1, 512], F32, tag="pj", bufs=2, name=f"pc_{oc}")
                        for eb in range(DB):
                            nc.tensor.matmul(pc, r(bv_col[:, eb:eb + 1]),
                                             r(wo[:, eb, :]),
                                             start=(eb == 0), stop=(eb == DB - 1))
                        nc.vector.tensor_copy(c_sb[:, oc, :], pc)
                    for nb in range(DB):
                        po = ps.tile([P, 512], F32, tag="pj", bufs=2,
                                     name=f"po{sfx}_{oc}_{nb}")
                        for eb in range(H):
                            nc.tensor.matmul(
                                po,
                                r(hT[:, eb, nb * P:(nb + 1) * P]),
                                r(wo[:, eb, :]),
                                start=(eb == 0), stop=False)
                        nc.tensor.matmul(po, r(ones_row), r(c_sb[:, oc, :]),
                                         start=False, stop=True)
                        osb = work.tile([P, 512], F32, name=f"o{sfx}_{oc}_{nb}",
                                        tag="osb", bufs=2)
                        nc.scalar.activation(osb, po,
                                             mybir.ActivationFunctionType.Copy)
                        nc.sync.dma_start(
                            out=out[b, nb * P:(nb + 1) * P, oc * 512:(oc + 1) * 512],
                            in_=osb)

    nc.compile()
    return nc


_NC_CACHE = None


def _get_nc():
    global _NC_CACHE
    if _NC_CACHE is None:
        _NC_CACHE = build_nc()
    return _NC_CACHE


def make_in_maps(x, Wq, bq, Wk, bk, Wv, bv, Wo):
    x = np.asarray(x, dtype=np.float32)
    in_maps = []
    shared = {
        "Wq": np.ascontiguousarray(Wq, dtype=np.float32),
        "Wk": np.ascontiguousarray(Wk, dtype=np.float32),
        "Wv": np.ascontiguousarray(Wv, dtype=np.float32),
        "Wo": np.ascontiguousarray(Wo, dtype=np.float32),
        "bq": np.ascontiguousarray(bq, dtype=np.float32),
        "bk": np.ascontiguousarray(bk, dtype=np.float32),
        "bv": np.ascontiguousarray(bv, dtype=np.float32),
    }
    for c in range(N_CORES):
        xc = x[c * BPC:(c + 1) * BPC]                 # [BPC, N, D]
        xTc = np.ascontiguousarray(xc.transpose(0, 2, 1))  # [BPC, D, N]
        in_maps.append({"xT": xTc, **shared})
    return in_maps


def run(x, Wq, bq, Wk, bk, Wv, bv, Wo, trace=False):
    nc = _get_nc()
    in_maps = make_in_maps(x, Wq, bq, Wk, bk, Wv, bv, Wo)
    res = run_bass_kernel_spmd(nc, in_maps, list(range(N_CORES)), trace=trace)
    out = np.concatenate([res.results[c]["out"] for c in range(N_CORES)], axis=0)
    return out, res


def kernel(x, Wq, bq, Wk, bk, Wv, bv, Wo):
    out, _ = run(x, Wq, bq, Wk, bk, Wv, bv, Wo, trace=False)
    return out


# revision 13
# speedup vs baseline: 1.3672x; 1.1067x over previous
"""Multi-head attention (B=16, N=1024, D=1024, H=8, dh=128) on 8 trn2 cores.

Strategy: data-parallel over batch (2 batches/core), fp32r matmuls.
Per batch on each core:
  phase 1 (per 2-head group g): Q^T_g, K^T_g (head-transposed: dh on
    partitions) and V_g (natural) via fp32r matmuls from x^T (host-side
    pre-transposed) and streamed weight slices.
  phase 2 (per head, per 512-wide q chunk): S^T = K_h^T.T @ Q_h^T (k on
    partitions), E^T = exp(norm*S^T) on ACT, heads^T += V_h.T @ E^T, and
    R = colsum(E^T) via DVE/GpSimd adds + ones-vector matmul; 1/R
    broadcast to all partitions with a K=1 outer-product matmul.
  phase 3: out = (heads^T).T @ Wo in natural layout, scaled by 1/R via
    the broadcast tile, plus bv@Wo row via a K=1 matmul.
"""

import numpy as np

import concourse.bass as bass
import concourse.mybir as mybir
import concourse.tile as tile
from concourse import bacc
from concourse.bass_utils import run_bass_kernel_spmd

N_CORES = 8
B = 16
BPC = B // N_CORES      # batches per core
N = 1024                # sequence length
D = 1024                # model dim
H = 8                   # heads
DH = 128                # head dim
P = 128
DB = D // P             # 8 contraction blocks
GH = 2                  # heads per group
G = H // GH             # 4 groups
GW = GH * DH            # 256: e-width per group
NC2 = N // 512          # 2 n-chunks of 512
NORM = 1.0 / np.sqrt(DH)

F32 = mybir.dt.float32
F32R = mybir.dt.float32r


def r(ap):
    return ap


def build_nc():
    nc = bacc.Bacc()
    xT = nc.declare_dram_parameter("xT", [BPC, D, N], F32R, isOutput=False)
    Wq = nc.declare_dram_parameter("Wq", [D, D], F32R, isOutput=False)
    Wk = nc.declare_dram_parameter("Wk", [D, D], F32R, isOutput=False)
    Wv = nc.declare_dram_parameter("Wv", [D, D], F32R, isOutput=False)
    Wo = nc.declare_dram_parameter("Wo", [D, D], F32R, isOutput=False)
    bq = nc.declare_dram_parameter("bq", [D], F32, isOutput=False)
    bk = nc.declare_dram_parameter("bk", [D], F32, isOutput=False)
    bv = nc.declare_dram_parameter("bv", [D], F32R, isOutput=False)
    out = nc.declare_dram_parameter("out", [BPC, N, D], F32, isOutput=True)

    ws = [Wq, Wk, Wv]

    with tile.TileContext(nc) as tc:
        with tc.tile_pool(name="big", bufs=1) as big, \
             tc.tile_pool(name="wp", bufs=1) as wp, \
             tc.tile_pool(name="work", bufs=1) as work, \
             tc.tile_pool(name="small", bufs=1) as small, \
             tc.tile_pool(name="ps", bufs=1, space="PSUM") as ps:

            # constants / biases
            bq_col = small.tile([P, DB], F32, name="bq_col")
            bk_col = small.tile([P, DB], F32, name="bk_col")
            bv_col = small.tile([P, DB], F32R, name="bv_col")
            nc.sync.dma_start(out=bq_col, in_=bq.rearrange("(eb p) -> p eb", p=P))
            nc.sync.dma_start(out=bk_col, in_=bk.rearrange("(eb p) -> p eb", p=P))
            nc.sync.dma_start(out=bv_col, in_=bv.rearrange("(eb p) -> p eb", p=P))
            ones_f32 = small.tile([P, 1], F32, name="ones_f32")
            nc.vector.memset(ones_f32, 1.0)
            ones_row_f32 = small.tile([1, P], F32, name="ones_row_f32")
            nc.vector.memset(ones_row_f32, 1.0)
            ones_col = small.tile([P, 1], F32R, name="ones_col")
            nc.vector.tensor_copy(ones_col, ones_f32)
            ones_row = small.tile([1, P], F32R, name="ones_row")
            nc.vector.tensor_copy(ones_row, ones_row_f32)
            c_sb = small.tile([1, NC2, 512], F32R, name="c_sb")

            def emit_proj_unit(b, g, kind, idx, xt, wgt, qTg, kTg, vg):
                """Emit one psum accumulation group of phase 1."""
                gsfx = f"_b{b}_g{g}"
                e0 = g * GW
                if kind in ("q", "k"):
                    dst, wt, bcol = ((qTg, wgt["wq"], bq_col) if kind == "q"
                                     else (kTg, wgt["wk"], bk_col))
                    eb, nch = divmod(idx, NC2)
                    acc = ps.tile([P, 512], F32, tag="pj", bufs=2,
                                  name=f"p{kind}{gsfx}_{eb}_{nch}")
                    for db in range(DB):
                        nc.tensor.matmul(
                            acc,
                            r(wt[:, db, eb * P:(eb + 1) * P]),
                            r(xt[:, db, nch * 512:(nch + 1) * 512]),
                            start=(db == 0), stop=(db == DB - 1))
                    ebg = (e0 // P) + eb
                    nc.vector.tensor_scalar_add(
                        dst[:, eb, nch * 512:(nch + 1) * 512],
                        acc, bcol[:, ebg:ebg + 1])
                else:  # "v"
                    nb = idx
                    accv = ps.tile([P, 512], F32, tag="pj", bufs=2,
                                   name=f"pv{gsfx}_{nb}")
                    for db in range(DB):
                        nc.tensor.matmul(
                            accv[:, :GW],
                            r(xt[:, db, nb * P:(nb + 1) * P]),
                            r(wgt["wv"][:, db, :]),
                            start=(db == 0), stop=(db == DB - 1))
                    nc.vector.tensor_copy(vg[:, nb, :], accv[:, :GW])

            for b in range(BPC):
                sfx = f"_b{b}"
                # ---- load x^T for this batch: [128, db, n]
                xt = big.tile([P, DB, N], F32R, name=f"xt{sfx}", tag="xt")
                nc.sync.dma_start(out=xt, in_=xT[b].rearrange("(db p) n -> p db n", p=P))

                hT = big.tile([P, H, N], F32R, name=f"hT{sfx}", tag="hT")

                # pending attention units from the previous group, emitted
                # interleaved with the next group's projection units so PE
                # always has ready matmuls during exp waits
                attn_queue = []

                for g in range(G):
                    gsfx = f"{sfx}_g{g}"
                    e0 = g * GW
                    # ---- weight slices for this group: [128, db, GW]
                    wgt = {}
                    for wi, wname in enumerate(("wq", "wk", "wv")):
                        wt = wp.tile([P, DB, GW], F32R, name=f"{wname}{gsfx}", tag="wg", bufs=2)
                        src = ws[wi].rearrange("(db p) e -> p db e", p=P)
                        nc.sync.dma_start(out=wt, in_=src[:, :, e0:e0 + GW])
                        wgt[wname] = wt

                    qTg = work.tile([P, GH, N], F32R, name=f"qT{gsfx}", tag="qTg", bufs=2)
                    kTg = work.tile([P, GH, N], F32R, name=f"kT{gsfx}", tag="kTg", bufs=2)
                    vg = work.tile([P, DB, GW], F32R, name=f"v{gsfx}", tag="vg", bufs=2)

                    # 16 proj units: 4 Q, 4 K, 8 V; interleave with up to 4
                    # pending attention units (1 attention per 4 proj units)
                    units = ([("q", i) for i in range(GH * NC2)]
                             + [("k", i) for i in range(GH * NC2)]
                             + [("v", i) for i in range(DB)])
                    for ui, (kind, idx) in enumerate(units):
                        emit_proj_unit(b, g, kind, idx, xt, wgt, qTg, kTg, vg)
                        if ui % 4 == 3 and attn_queue:
                            attn_queue.pop(0)()
                    while attn_queue:
                        attn_queue.pop(0)()

                    # ---- queue attention for the heads of this group
                    def make_attn(g, hh, qc, qTg=qTg, kTg=kTg, vg=vg, hT=hT, b=b):
                        def emit():
                            h = g * GH + hh
                            asfx = f"_b{b}_h{h}_q{qc}"
                            eT = work.tile([P, 4, 1024], F32R, name=f"eT{asfx}",
                                           tag="eT", bufs=2)
                            for j in range(4):
                                # scores for kb=2j, 2j+1 into one 2-bank tile
                                sp = ps.tile([P, 1024], F32, tag="spair", bufs=2,
                                             name=f"sp{asfx}_{j}")
                                for half in range(2):
                                    kb = 2 * j + half
                                    nc.tensor.matmul(
                                        sp[:, half * 512:(half + 1) * 512],
                                        r(kTg[:, hh, kb * P:(kb + 1) * P]),
                                        r(qTg[:, hh, qc * 512:(qc + 1) * 512]),
                                        start=True, stop=True)
                                nc.scalar.activation(
                                    eT[:, j, :], sp,
                                    mybir.ActivationFunctionType.Exp,
                                    scale=float(NORM))

                            # heads^T (unnormalized): [dv(128) x q(512)]
                            pav = ps.tile([P, 512], F32, tag="pav", bufs=1,
                                          name=f"pav{asfx}")
                            for j in range(4):
                                for half in range(2):
                                    kb = 2 * j + half
                                    nc.tensor.matmul(
                                        pav,
                                        r(vg[:, kb, hh * DH:(hh + 1) * DH]),
                                        r(eT[:, j, half * 512:(half + 1) * 512]),
                                        start=(kb == 0), stop=(kb == DB - 1))

                            # R = col-sum of E^T: pairwise adds split DVE/GpSimd
                            add = mybir.AluOpType.add
                            tA = work.tile([P, 1024], F32R, name=f"tA{asfx}", tag="tA", bufs=1)
                            tB = work.tile([P, 1024], F32R, name=f"tB{asfx}", tag="tB", bufs=1)
                            rp = work.tile([P, 512], F32R, name=f"rp{asfx}", tag="rp", bufs=2)
                            nc.vector.tensor_tensor(tA, eT[:, 0, :], eT[:, 1, :], add)
                            nc.vector.tensor_tensor(tB, eT[:, 2, :], eT[:, 3, :], add)
                            nc.vector.tensor_tensor(tA, tA, tB, add)
                            nc.vector.tensor_tensor(rp, tA[:, 0:512], tA[:, 512:1024], add)
                            pr = ps.tile([1, 512], F32, tag="pnorm", bufs=1, name=f"pr{asfx}")
                            nc.tensor.matmul(pr, r(ones_col), r(rp),
                                             start=True, stop=True)
                            # R row to SBUF (rounded to fp32r) then broadcast to
                            # all partitions via a K=1 outer-product matmul
                            r_row = small.tile([1, 512], F32R, name=f"rrow{asfx}",
                                               tag="rinv", bufs=1)
                            nc.vector.tensor_copy(r_row, pr)
                            pbc = ps.tile([P, 512], F32, tag="pnorm", bufs=1, name=f"pbc{asfx}")
                            nc.tensor.matmul(pbc, r(ones_row), r(r_row),
                                             start=True, stop=True)
                            # 1/R at full 128-lane width (approx + one NR pass)
                            scratch = work.tile([P, 512], F32, name=f"sc{asfx}",
                                                tag="bc", bufs=1)
                            binv = work.tile([P, 512], F32, name=f"binv{asfx}",
                                             tag="binv", bufs=1)
                            nc.vector.reciprocal_approx_accurate(binv, pbc, scratch)
                            nc.vector.tensor_tensor(
                                hT[:, h, qc * 512:(qc + 1) * 512], pav, binv,
                                mybir.AluOpType.mult)
                        return emit

                    for hh in range(GH):
                        for qc in range(NC2):
                            attn_queue.append(make_attn(g, hh, qc))

                # drain the last group's attention before the output projection
                while attn_queue:
                    attn_queue.pop(0)()

                # ---- phase 3: out = heads_norm @ Wo + bv@Wo (natural layout)
                for oc in range(NC2):
                    wo = wp.tile([P, DB, 512], F32R, name=f"wo{sfx}_{oc}", tag="wo")
                    src = Wo.rearrange("(eb p) o -> p eb o", p=P)
                    nc.sync.dma_start(out=wo, in_=src[:, :, oc * 512:(oc + 1) * 512])
                    if b == 0:
                        # c = bv @ Wo for this o-chunk (once; reused for b=1)
                        pc = ps.tile([1, 512], F32, tag="pj", bufs=2, name=f"pc_{oc}")
                        for eb in range(DB):
                            nc.tensor.matmul(pc, r(bv_col[:, eb:eb + 1]),
                                             r(wo[:, eb, :]),
                                             start=(eb == 0), stop=(eb == DB - 1))
                        nc.vector.tensor_copy(c_sb[:, oc, :], pc)
                    for nb in range(DB):
                        po = ps.tile([P, 512], F32, tag="pj", bufs=2,
                                     name=f"po{sfx}_{oc}_{nb}")
                        for eb in range(H):
                            nc.tensor.matmul(
                                po,
                                r(hT[:, eb, nb * P:(nb + 1) * P]),
                                r(wo[:, eb, :]),
                                start=(eb == 0), stop=False)
                        nc.tensor.matmul(po, r(ones_row), r(c_sb[:, oc, :]),
                                         start=False, stop=True)
                        osb = work.tile([P, 512], F32, name=f"o{sfx}_{oc}_{nb}",
                                        tag="osb", bufs=2)
                        nc.scalar.activation(osb, po,
                                             mybir.ActivationFunctionType.Copy)
                        nc.sync.dma_start(
                            out=out[b, nb * P:(nb + 1) * P, oc * 512:(oc + 1) * 512],
                            in_=osb)

    nc.compile()
    return nc


_NC_CACHE = None


def _get_nc():
    global _NC_CACHE
    if _NC_CACHE is None:
        _NC_CACHE = build_nc()
    return _NC_CACHE


def make_in_maps(x, Wq, bq, Wk, bk, Wv, bv, Wo):
    x = np.asarray(x, dtype=np.float32)
    in_maps = []
    shared = {
        "Wq": np.ascontiguousarray(Wq, dtype=np.float32),
        "Wk": np.ascontiguousarray(Wk, dtype=np.float32),
        "Wv": np.ascontiguousarray(Wv, dtype=np.float32),
        "Wo": np.ascontiguousarray(Wo, dtype=np.float32),
        "bq": np.ascontiguousarray(bq, dtype=np.float32),
        "bk": np.ascontiguousarray(bk, dtype=np.float32),
        "bv": np.ascontiguousarray(bv, dtype=np.float32),
    }
    for c in range(N_CORES):
        xc = x[c * BPC:(c + 1) * BPC]                 # [BPC, N, D]
        xTc = np.ascontiguousarray(xc.transpose(0, 2, 1))  # [BPC, D, N]
        in_maps.append({"xT": xTc, **shared})
    return in_maps


def run(x, Wq, bq, Wk, bk, Wv, bv, Wo, trace=False):
    nc = _get_nc()
    in_maps = make_in_maps(x, Wq, bq, Wk, bk, Wv, bv, Wo)
    res = run_bass_kernel_spmd(nc, in_maps, list(range(N_CORES)), trace=trace)
    out = np.concatenate([res.results[c]["out"] for c in range(N_CORES)], axis=0)
    return out, res


def kernel(x, Wq, bq, Wk, bk, Wv, bv, Wo):
    out, _ = run(x, Wq, bq, Wk, bk, Wv, bv, Wo, trace=False)
    return out


# revision 16
# speedup vs baseline: 1.3839x; 1.0122x over previous
"""Multi-head attention (B=16, N=1024, D=1024, H=8, dh=128) on 8 trn2 cores.

Strategy: data-parallel over batch (2 batches/core), fp32r matmuls.
Per batch on each core:
  phase 1 (per 2-head group g): Q^T_g, K^T_g (head-transposed: dh on
    partitions) and V_g (natural) via fp32r matmuls from x^T (host-side
    pre-transposed) and streamed weight slices.
  phase 2 (per head, per 512-wide q chunk): S^T = K_h^T.T @ Q_h^T (k on
    partitions), E^T = exp(norm*S^T) on ACT, heads^T += V_h.T @ E^T, and
    R = colsum(E^T) via DVE/GpSimd adds + ones-vector matmul; 1/R
    broadcast to all partitions with a K=1 outer-product matmul.
  phase 3: out = (heads^T).T @ Wo in natural layout, scaled by 1/R via
    the broadcast tile, plus bv@Wo row via a K=1 matmul.
"""

import numpy as np

import concourse.bass as bass
import concourse.mybir as mybir
import concourse.tile as tile
from concourse import bacc
from concourse.bass_utils import run_bass_kernel_spmd

N_CORES = 8
B = 16
BPC = B // N_CORES      # batches per core
N = 1024                # sequence length
D = 1024                # model dim
H = 8                   # heads
DH = 128                # head dim
P = 128
DB = D // P             # 8 contraction blocks
GH = 2                  # heads per group
G = H // GH             # 4 groups
GW = GH * DH            # 256: e-width per group
NC2 = N // 512          # 2 n-chunks of 512
NORM = 1.0 / np.sqrt(DH)

F32 = mybir.dt.float32
F32R = mybir.dt.float32r


def r(ap):
    return ap


def build_nc():
    nc = bacc.Bacc()
    xT = nc.declare_dram_parameter("xT", [BPC, D, N], F32R, isOutput=False)
    Wq = nc.declare_dram_parameter("Wq", [D, D], F32R, isOutput=False)
    Wk = nc.declare_dram_parameter("Wk", [D, D], F32R, isOutput=False)
    Wv = nc.declare_dram_parameter("Wv", [D, D], F32R, isOutput=False)
    Wo = nc.declare_dram_parameter("Wo", [D, D], F32R, isOutput=False)
    bq = nc.declare_dram_parameter("bq", [D], F32, isOutput=False)
    bk = nc.declare_dram_parameter("bk", [D], F32, isOutput=False)
    bv = nc.declare_dram_parameter("bv", [D], F32R, isOutput=False)
    out = nc.declare_dram_parameter("out", [BPC, N, D], F32, isOutput=True)

    ws = [Wq, Wk, Wv]

    with tile.TileContext(nc) as tc:
        with tc.tile_pool(name="big", bufs=1) as big, \
             tc.tile_pool(name="wp", bufs=1) as wp, \
             tc.tile_pool(name="work", bufs=1) as work, \
             tc.tile_pool(name="small", bufs=1) as small, \
             tc.tile_pool(name="ps", bufs=1, space="PSUM") as ps:

            # constants / biases
            bq_col = small.tile([P, DB], F32, name="bq_col")
            bk_col = small.tile([P, DB], F32, name="bk_col")
            bv_col = small.tile([P, DB], F32R, name="bv_col")
            nc.sync.dma_start(out=bq_col, in_=bq.rearrange("(eb p) -> p eb", p=P))
            nc.sync.dma_start(out=bk_col, in_=bk.rearrange("(eb p) -> p eb", p=P))
            nc.sync.dma_start(out=bv_col, in_=bv.rearrange("(eb p) -> p eb", p=P))
            ones_f32 = small.tile([P, 1], F32, name="ones_f32")
            nc.vector.memset(ones_f32, 1.0)
            ones_row_f32 = small.tile([1, P], F32, name="ones_row_f32")
            nc.vector.memset(ones_row_f32, 1.0)
            ones_col = small.tile([P, 1], F32R, name="ones_col")
            nc.vector.tensor_copy(ones_col, ones_f32)
            ones_row = small.tile([1, P], F32R, name="ones_row")
            nc.vector.tensor_copy(ones_row, ones_row_f32)
            c_sb = small.tile([1, NC2, 512], F32R, name="c_sb")

            def emit_proj_unit(b, g, kind, idx, xt, wgt, qTg, kTg, vg):
                """Emit one psum accumulation group of phase 1."""
                gsfx = f"_b{b}_g{g}"
                e0 = g * GW
                if kind in ("q", "k"):
                    dst, wt, bcol = ((qTg, wgt["wq"], bq_col) if kind == "q"
                                     else (kTg, wgt["wk"], bk_col))
                    eb, nch = divmod(idx, NC2)
                    acc = ps.tile([P, 512], F32, tag="pj", bufs=2,
                                  name=f"p{kind}{gsfx}_{eb}_{nch}")
                    for db in range(DB):
                        nc.tensor.matmul(
                            acc,
                            r(wt[:, db, eb * P:(eb + 1) * P]),
                            r(xt[:, db, nch * 512:(nch + 1) * 512]),
                            start=(db == 0), stop=(db == DB - 1))
                    ebg = (e0 // P) + eb
                    nc.vector.tensor_scalar_add(
                        dst[:, eb, nch * 512:(nch + 1) * 512],
                        acc, bcol[:, ebg:ebg + 1])
                else:  # "v"
                    nb = idx
                    accv = ps.tile([P, 512], F32, tag="pj", bufs=2,
                                   name=f"pv{gsfx}_{nb}")
                    for db in range(DB):
                        nc.tensor.matmul(
                            accv[:, :GW],
                            r(xt[:, db, nb * P:(nb + 1) * P]),
                            r(wgt["wv"][:, db, :]),
                            start=(db == 0), stop=(db == DB - 1))
                    nc.vector.tensor_copy(vg[:, nb, :], accv[:, :GW])

            def make_phase3(b, hT):
                def emit():
                    sfx = f"_b{b}"
                    for oc in range(NC2):
                        wo = wp.tile([P, DB, 512], F32R, name=f"wo{sfx}_{oc}", tag="wo")
                        src = Wo.rearrange("(eb p) o -> p eb o", p=P)
                        nc.sync.dma_start(out=wo, in_=src[:, :, oc * 512:(oc + 1) * 512])
                        if b == 0:
                            # c = bv @ Wo for this o-chunk (once; reused for b=1)
                            pc = ps.tile([1, 512], F32, tag="pj", bufs=2, name=f"pc_{oc}")
                            for eb in range(DB):
                                nc.tensor.matmul(pc, r(bv_col[:, eb:eb + 1]),
                                                 r(wo[:, eb, :]),
                                                 start=(eb == 0), stop=(eb == DB - 1))
                            nc.vector.tensor_copy(c_sb[:, oc, :], pc)
                        for nb in range(DB):
                            po = ps.tile([P, 512], F32, tag="pj", bufs=2,
                                         name=f"po{sfx}_{oc}_{nb}")
                            for eb in range(H):
                                nc.tensor.matmul(
                                    po,
                                    r(hT[:, eb, nb * P:(nb + 1) * P]),
                                    r(wo[:, eb, :]),
                                    start=(eb == 0), stop=False)
                            nc.tensor.matmul(po, r(ones_row), r(c_sb[:, oc, :]),
                                             start=False, stop=True)
                            osb = work.tile([P, 512], F32, name=f"o{sfx}_{oc}_{nb}",
                                            tag="osb", bufs=2)
                            nc.scalar.activation(osb, po,
                                                 mybir.ActivationFunctionType.Copy)
                            nc.sync.dma_start(
                                out=out[b, nb * P:(nb + 1) * P, oc * 512:(oc + 1) * 512],
                                in_=osb)
                return emit

            # attention units and the previous batch's output projection are
            # emitted interleaved with later projection units so PE always
            # has ready matmuls during exp/epilogue waits
            attn_queue = []
            pending_phase3 = None

            for b in range(BPC):
                sfx = f"_b{b}"
                # ---- load x^T for this batch: [128, db, n]
                xt = big.tile([P, DB, N], F32R, name=f"xt{sfx}", tag="xt")
                nc.sync.dma_start(out=xt, in_=xT[b].rearrange("(db p) n -> p db n", p=P))

                hT = None

                for g in range(G):
                    gsfx = f"{sfx}_g{g}"
                    e0 = g * GW
                    # ---- weight slices for this group: [128, db, GW]
                    wgt = {}
                    for wi, wname in enumerate(("wq", "wk", "wv")):
                        wt = wp.tile([P, DB, GW], F32R, name=f"{wname}{gsfx}", tag="wg", bufs=2)
                        src = ws[wi].rearrange("(db p) e -> p db e", p=P)
                        nc.sync.dma_start(out=wt, in_=src[:, :, e0:e0 + GW])
                        wgt[wname] = wt

                    qTg = work.tile([P, GH, N], F32R, name=f"qT{gsfx}", tag="qTg", bufs=2)
                    kTg = work.tile([P, GH, N], F32R, name=f"kT{gsfx}", tag="kTg", bufs=2)
                    vg = work.tile([P, DB, GW], F32R, name=f"v{gsfx}", tag="vg", bufs=2)

                    # 16 proj units: 4 Q, 4 K, 8 V; interleave with up to 4
                    # pending attention units (1 attention per 4 proj units)
                    units = ([("q", i) for i in range(GH * NC2)]
                             + [("k", i) for i in range(GH * NC2)]
                             + [("v", i) for i in range(DB)])
                    for ui, (kind, idx) in enumerate(units):
                        emit_proj_unit(b, g, kind, idx, xt, wgt, qTg, kTg, vg)
                        if ui % 4 == 3 and attn_queue:
                            attn_queue.pop(0)()
                    while attn_queue:
                        attn_queue.pop(0)()
                    if pending_phase3 is not None:
                        pending_phase3()
                        pending_phase3 = None
                    if hT is None:
                        hT = big.tile([P, H, N], F32R, name=f"hT{sfx}", tag="hT")

                    # ---- queue attention for the heads of this group
                    def make_attn(g, hh, qc, qTg=qTg, kTg=kTg, vg=vg, hT=hT, b=b):
                        def emit():
                            h = g * GH + hh
                            asfx = f"_b{b}_h{h}_q{qc}"
                            eT = work.tile([P, 4, 1024], F32R, name=f"eT{asfx}",
                                           tag="eT", bufs=2)
                            for j in range(4):
                                # scores for kb=2j, 2j+1 into one 2-bank tile
                                sp = ps.tile([P, 1024], F32, tag="spair", bufs=2,
                                             name=f"sp{asfx}_{j}")
                                for half in range(2):
                                    kb = 2 * j + half
                                    nc.tensor.matmul(
                                        sp[:, half * 512:(half + 1) * 512],
                                        r(kTg[:, hh, kb * P:(kb + 1) * P]),
                                        r(qTg[:, hh, qc * 512:(qc + 1) * 512]),
                                        start=True, stop=True)
                                nc.scalar.activation(
                                    eT[:, j, :], sp,
                                    mybir.ActivationFunctionType.Exp,
                                    scale=float(NORM))

                            # heads^T (unnormalized): [dv(128) x q(512)]
                            pav = ps.tile([P, 512], F32, tag="pav", bufs=1,
                                          name=f"pav{asfx}")
                            for j in range(4):
                                for half in range(2):
                                    kb = 2 * j + half
                                    nc.tensor.matmul(
                                        pav,
                                        r(vg[:, kb, hh * DH:(hh + 1) * DH]),
                                        r(eT[:, j, half * 512:(half + 1) * 512]),
                                        start=(kb == 0), stop=(kb == DB - 1))

                            # R = col-sum of E^T: pairwise adds split DVE/GpSimd
                            add = mybir.AluOpType.add
                            tA = work.tile([P, 1024], F32R, name=f"tA{asfx}", tag="tA", bufs=1)
                            tB = work.tile([P, 1024], F32R, name=f"tB{asfx}", tag="tB", bufs=1)
                            rp = work.tile([P, 512], F32R, name=f"rp{asfx}", tag="rp", bufs=2)
                            nc.vector.tensor_tensor(tA, eT[:, 0, :], eT[:, 1, :], add)
                            nc.vector.tensor_tensor(tB, eT[:, 2, :], eT[:, 3, :], add)
                            nc.vector.tensor_tensor(tA, tA, tB, add)
                            nc.vector.tensor_tensor(rp, tA[:, 0:512], tA[:, 512:1024], add)
                            pr = ps.tile([1, 512], F32, tag="pnorm", bufs=1, name=f"pr{asfx}")
                            nc.tensor.matmul(pr, r(ones_col), r(rp),
                                             start=True, stop=True)
                            # R row to SBUF (rounded to fp32r) then broadcast to
                            # all partitions via a K=1 outer-product matmul
                            r_row = small.tile([1, 512], F32R, name=f"rrow{asfx}",
                                               tag="rinv", bufs=1)
                            nc.vector.tensor_copy(r_row, pr)
                            pbc = ps.tile([P, 512], F32, tag="pnorm", bufs=1, name=f"pbc{asfx}")
                            nc.tensor.matmul(pbc, r(ones_row), r(r_row),
                                             start=True, stop=True)
                            # 1/R at full 128-lane width (approx + one NR pass)
                            scratch = work.tile([P, 512], F32, name=f"sc{asfx}",
                                                tag="bc", bufs=1)
                            binv = work.tile([P, 512], F32, name=f"binv{asfx}",
                                             tag="binv", bufs=1)
                            nc.vector.reciprocal_approx_accurate(binv, pbc, scratch)
                            nc.vector.tensor_tensor(
                                hT[:, h, qc * 512:(qc + 1) * 512], pav, binv,
                                mybir.AluOpType.mult)
                        return emit

                    for hh in range(GH):
                        for qc in range(NC2):
                            attn_queue.append(make_attn(g, hh, qc))

                # phase 3 of this batch is deferred: it is emitted after the
                # next batch's first projection group so its matmuls overlap
                # the last attention units
                pending_phase3 = make_phase3(b, hT)

            # tail: drain remaining attention, then the last output projection
            while attn_queue:
                attn_queue.pop(0)()
            if pending_phase3 is not None:
                pending_phase3()

    nc.compile()
    return nc


_NC_CACHE = None


def _get_nc():
    global _NC_CACHE
    if _NC_CACHE is None:
        _NC_CACHE = build_nc()
    return _NC_CACHE


def make_in_maps(x, Wq, bq, Wk, bk, Wv, bv, Wo):
    x = np.asarray(x, dtype=np.float32)
    in_maps = []
    shared = {
        "Wq": np.ascontiguousarray(Wq, dtype=np.float32),
        "Wk": np.ascontiguousarray(Wk, dtype=np.float32),
        "Wv": np.ascontiguousarray(Wv, dtype=np.float32),
        "Wo": np.ascontiguousarray(Wo, dtype=np.float32),
        "bq": np.ascontiguousarray(bq, dtype=np.float32),
        "bk": np.ascontiguousarray(bk, dtype=np.float32),
        "bv": np.ascontiguousarray(bv, dtype=np.float32),
    }
    for c in range(N_CORES):
        xc = x[c * BPC:(c + 1) * BPC]                 # [BPC, N, D]
        xTc = np.ascontiguousarray(xc.transpose(0, 2, 1))  # [BPC, D, N]
        in_maps.append({"xT": xTc, **shared})
    return in_maps


def run(x, Wq, bq, Wk, bk, Wv, bv, Wo, trace=False):
    nc = _get_nc()
    in_maps = make_in_maps(x, Wq, bq, Wk, bk, Wv, bv, Wo)
    res = run_bass_kernel_spmd(nc, in_maps, list(range(N_CORES)), trace=trace)
    out = np.concatenate([res.results[c]["out"] for c in range(N_CORES)], axis=0)
    return out, res


def kernel(x, Wq, bq, Wk, bk, Wv, bv, Wo):
    out, _ = run(x, Wq, bq, Wk, bk, Wv, bv, Wo, trace=False)
    return out


# revision 17
# speedup vs baseline: 1.4048x; 1.0151x over previous
"""Multi-head attention (B=16, N=1024, D=1024, H=8, dh=128) on 8 trn2 cores.

Strategy: data-parallel over batch (2 batches/core), fp32r matmuls.
Per batch on each core:
  phase 1 (per 2-head group g): Q^T_g, K^T_g (head-transposed: dh on
    partitions) and V_g (natural) via fp32r matmuls from x^T (host-side
    pre-transposed) and streamed weight slices.
  phase 2 (per head, per 512-wide q chunk): S^T = K_h^T.T @ Q_h^T (k on
    partitions), E^T = exp(norm*S^T) on ACT, heads^T += V_h.T @ E^T, and
    R = colsum(E^T) via DVE/GpSimd adds + ones-vector matmul; 1/R
    broadcast to all partitions with a K=1 outer-product matmul.
  phase 3: out = (heads^T).T @ Wo in natural layout, scaled by 1/R via
    the broadcast tile, plus bv@Wo row via a K=1 matmul.
"""

import numpy as np

import concourse.bass as bass
import concourse.mybir as mybir
import concourse.tile as tile
from concourse import bacc
from concourse.bass_utils import run_bass_kernel_spmd

N_CORES = 8
B = 16
BPC = B // N_CORES      # batches per core
N = 1024                # sequence length
D = 1024                # model dim
H = 8                   # heads
DH = 128                # head dim
P = 128
DB = D // P             # 8 contraction blocks
GH = 2                  # heads per group
G = H // GH             # 4 groups
GW = GH * DH            # 256: e-width per group
NC2 = N // 512          # 2 n-chunks of 512
NORM = 1.0 / np.sqrt(DH)

F32 = mybir.dt.float32
F32R = mybir.dt.float32r


def r(ap):
    return ap


def build_nc():
    nc = bacc.Bacc()
    xT = nc.declare_dram_parameter("xT", [BPC, D, N], F32R, isOutput=False)
    Wq = nc.declare_dram_parameter("Wq", [D, D], F32R, isOutput=False)
    Wk = nc.declare_dram_parameter("Wk", [D, D], F32R, isOutput=False)
    Wv = nc.declare_dram_parameter("Wv", [D, D], F32R, isOutput=False)
    Wo = nc.declare_dram_parameter("Wo", [D, D], F32R, isOutput=False)
    bq = nc.declare_dram_parameter("bq", [D], F32, isOutput=False)
    bk = nc.declare_dram_parameter("bk", [D], F32, isOutput=False)
    bv = nc.declare_dram_parameter("bv", [D], F32R, isOutput=False)
    out = nc.declare_dram_parameter("out", [BPC, N, D], F32, isOutput=True)

    ws = [Wq, Wk, Wv]

    with tile.TileContext(nc) as tc:
        with tc.tile_pool(name="big", bufs=1) as big, \
             tc.tile_pool(name="wp", bufs=1) as wp, \
             tc.tile_pool(name="work", bufs=1) as work, \
             tc.tile_pool(name="small", bufs=1) as small, \
             tc.tile_pool(name="ps", bufs=1, space="PSUM") as ps:

            # constants / biases
            bq_col = small.tile([P, DB], F32, name="bq_col")
            bk_col = small.tile([P, DB], F32, name="bk_col")
            bv_col = small.tile([P, DB], F32R, name="bv_col")
            nc.sync.dma_start(out=bq_col, in_=bq.rearrange("(eb p) -> p eb", p=P))
            nc.sync.dma_start(out=bk_col, in_=bk.rearrange("(eb p) -> p eb", p=P))
            nc.sync.dma_start(out=bv_col, in_=bv.rearrange("(eb p) -> p eb", p=P))
            ones_f32 = small.tile([P, 1], F32, name="ones_f32")
            nc.vector.memset(ones_f32, 1.0)
            ones_row_f32 = small.tile([1, P], F32, name="ones_row_f32")
            nc.vector.memset(ones_row_f32, 1.0)
            ones_col = small.tile([P, 1], F32R, name="ones_col")
            nc.vector.tensor_copy(ones_col, ones_f32)
            ones_row = small.tile([1, P], F32R, name="ones_row")
            nc.vector.tensor_copy(ones_row, ones_row_f32)
            c_sb = small.tile([1, NC2, 512], F32R, name="c_sb")

            def emit_proj_unit(b, g, kind, idx, xt, wgt, qTg, kTg, vg):
                """Emit one psum accumulation group of phase 1."""
                gsfx = f"_b{b}_g{g}"
                e0 = g * GW
                if kind in ("q", "k"):
                    dst, wt, bcol = ((qTg, wgt["wq"], bq_col) if kind == "q"
                                     else (kTg, wgt["wk"], bk_col))
                    eb, nch = divmod(idx, NC2)
                    acc = ps.tile([P, 512], F32, tag="pj", bufs=2,
                                  name=f"p{kind}{gsfx}_{eb}_{nch}")
                    for db in range(DB):
                        nc.tensor.matmul(
                            acc,
                            r(wt[:, db, eb * P:(eb + 1) * P]),
                            r(xt[:, db, nch * 512:(nch + 1) * 512]),
                            start=(db == 0), stop=(db == DB - 1))
                    ebg = (e0 // P) + eb
                    nc.vector.tensor_scalar_add(
                        dst[:, eb, nch * 512:(nch + 1) * 512],
                        acc, bcol[:, ebg:ebg + 1])
                else:  # "v"
                    nb = idx
                    accv = ps.tile([P, 512], F32, tag="pj", bufs=2,
                                   name=f"pv{gsfx}_{nb}")
                    for db in range(DB):
                        nc.tensor.matmul(
                            accv[:, :GW],
                            r(xt[:, db, nb * P:(nb + 1) * P]),
                            r(wgt["wv"][:, db, :]),
                            start=(db == 0), stop=(db == DB - 1))
                    nc.vector.tensor_copy(vg[:, nb, :], accv[:, :GW])

            def make_phase3(b, hT):
                def emit():
                    sfx = f"_b{b}"
                    for oc in range(NC2):
                        wo = wp.tile([P, DB, 512], F32R, name=f"wo{sfx}_{oc}", tag="wo")
                        src = Wo.rearrange("(eb p) o -> p eb o", p=P)
                        nc.sync.dma_start(out=wo[:, 0:DB // 2, :],
                                          in_=src[:, 0:DB // 2, oc * 512:(oc + 1) * 512])
                        nc.sync.dma_start(out=wo[:, DB // 2:, :],
                                          in_=src[:, DB // 2:, oc * 512:(oc + 1) * 512])
                        if b == 0:
                            # c = bv @ Wo for this o-chunk (once; reused for b=1)
                            pc = ps.tile([1, 512], F32, tag="pj", bufs=2, name=f"pc_{oc}")
                            for eb in range(DB):
                                nc.tensor.matmul(pc, r(bv_col[:, eb:eb + 1]),
                                                 r(wo[:, eb, :]),
                                                 start=(eb == 0), stop=(eb == DB - 1))
                            nc.vector.tensor_copy(c_sb[:, oc, :], pc)
                        for nb in range(DB):
                            po = ps.tile([P, 512], F32, tag="pj", bufs=2,
                                         name=f"po{sfx}_{oc}_{nb}")
                            for eb in range(H):
                                nc.tensor.matmul(
                                    po,
                                    r(hT[:, eb, nb * P:(nb + 1) * P]),
                                    r(wo[:, eb, :]),
                                    start=(eb == 0), stop=False)
                            nc.tensor.matmul(po, r(ones_row), r(c_sb[:, oc, :]),
                                             start=False, stop=True)
                            osb = work.tile([P, 512], F32, name=f"o{sfx}_{oc}_{nb}",
                                            tag="osb", bufs=2)
                            nc.scalar.activation(osb, po,
                                                 mybir.ActivationFunctionType.Copy)
                            nc.sync.dma_start(
                                out=out[b, nb * P:(nb + 1) * P, oc * 512:(oc + 1) * 512],
                                in_=osb)
                return emit

            # attention units and the previous batch's output projection are
            # emitted interleaved with later projection units so PE always
            # has ready matmuls during exp/epilogue waits
            attn_queue = []
            pending_phase3 = None

            for b in range(BPC):
                sfx = f"_b{b}"
                # ---- load x^T for this batch: [128, db, n]
                xt = big.tile([P, DB, N], F32R, name=f"xt{sfx}", tag="xt")
                xsrc = xT[b].rearrange("(db p) n -> p db n", p=P)
                for db in range(DB):
                    nc.sync.dma_start(out=xt[:, db, :], in_=xsrc[:, db, :])

                hT = None

                for g in range(G):
                    gsfx = f"{sfx}_g{g}"
                    e0 = g * GW
                    # ---- weight slices for this group: [128, db, GW]
                    wgt = {}
                    for wi, wname in enumerate(("wq", "wk", "wv")):
                        wt = wp.tile([P, DB, GW], F32R, name=f"{wname}{gsfx}", tag="wg", bufs=2)
                        src = ws[wi].rearrange("(db p) e -> p db e", p=P)
                        nc.sync.dma_start(out=wt[:, 0:DB // 2, :],
                                          in_=src[:, 0:DB // 2, e0:e0 + GW])
                        nc.sync.dma_start(out=wt[:, DB // 2:, :],
                                          in_=src[:, DB // 2:, e0:e0 + GW])
                        wgt[wname] = wt

                    qTg = work.tile([P, GH, N], F32R, name=f"qT{gsfx}", tag="qTg", bufs=2)
                    kTg = work.tile([P, GH, N], F32R, name=f"kT{gsfx}", tag="kTg", bufs=2)
                    vg = work.tile([P, DB, GW], F32R, name=f"v{gsfx}", tag="vg", bufs=2)

                    # 16 proj units: 4 Q, 4 K, 8 V; interleave with up to 4
                    # pending attention units (1 attention per 4 proj units)
                    units = ([("q", i) for i in range(GH * NC2)]
                             + [("k", i) for i in range(GH * NC2)]
                             + [("v", i) for i in range(DB)])
                    for ui, (kind, idx) in enumerate(units):
                        emit_proj_unit(b, g, kind, idx, xt, wgt, qTg, kTg, vg)
                        if ui % 4 == 3 and attn_queue:
                            attn_queue.pop(0)()
                    while attn_queue:
                        attn_queue.pop(0)()
                    if pending_phase3 is not None:
                        pending_phase3()
                        pending_phase3 = None
                    if hT is None:
                        hT = big.tile([P, H, N], F32R, name=f"hT{sfx}", tag="hT")

                    # ---- queue attention for the heads of this group
                    def make_attn(g, hh, qc, qTg=qTg, kTg=kTg, vg=vg, hT=hT, b=b):
                        def emit():
                            h = g * GH + hh
                            asfx = f"_b{b}_h{h}_q{qc}"
                            eT = work.tile([P, 4, 1024], F32R, name=f"eT{asfx}",
                                           tag="eT", bufs=2)
                            for j in range(4):
                                # scores for kb=2j, 2j+1 into one 2-bank tile
                                sp = ps.tile([P, 1024], F32, tag="spair", bufs=2,
                                             name=f"sp{asfx}_{j}")
                                for half in range(2):
                                    kb = 2 * j + half
                                    nc.tensor.matmul(
                                        sp[:, half * 512:(half + 1) * 512],
                                        r(kTg[:, hh, kb * P:(kb + 1) * P]),
                                        r(qTg[:, hh, qc * 512:(qc + 1) * 512]),
                                        start=True, stop=True)
                                nc.scalar.activation(
                                    eT[:, j, :], sp,
                                    mybir.ActivationFunctionType.Exp,
                                    scale=float(NORM))

                            # heads^T (unnormalized): [dv(128) x q(512)]
                            pav = ps.tile([P, 512], F32, tag="pav", bufs=1,
                                          name=f"pav{asfx}")
                            for j in range(4):
                                for half in range(2):
                                    kb = 2 * j + half
                                    nc.tensor.matmul(
                                        pav,
                                        r(vg[:, kb, hh * DH:(hh + 1) * DH]),
                                        r(eT[:, j, half * 512:(half + 1) * 512]),
                                        start=(kb == 0), stop=(kb == DB - 1))

                            # R = col-sum of E^T: pairwise adds split DVE/GpSimd
                            add = mybir.AluOpType.add
                            tA = work.tile([P, 1024], F32R, name=f"tA{asfx}", tag="tA", bufs=1)
                            tB = work.tile([P, 1024], F32R, name=f"tB{asfx}", tag="tB", bufs=1)
                            rp = work.tile([P, 512], F32R, name=f"rp{asfx}", tag="rp", bufs=2)
                            nc.vector.tensor_tensor(tA, eT[:, 0, :], eT[:, 1, :], add)
                            nc.vector.tensor_tensor(tB, eT[:, 2, :], eT[:, 3, :], add)
                            nc.vector.tensor_tensor(tA, tA, tB, add)
                            nc.vector.tensor_tensor(rp, tA[:, 0:512], tA[:, 512:1024], add)
                            pr = ps.tile([1, 512], F32, tag="pnorm", bufs=1, name=f"pr{asfx}")
                            nc.tensor.matmul(pr, r(ones_col), r(rp),
                                             start=True, stop=True)
                            # R row to SBUF (rounded to fp32r) then broadcast to
                            # all partitions via a K=1 outer-product matmul
                            r_row = small.tile([1, 512], F32R, name=f"rrow{asfx}",
                                               tag="rinv", bufs=1)
                            nc.vector.tensor_copy(r_row, pr)
                            pbc = ps.tile([P, 512], F32, tag="pnorm", bufs=1, name=f"pbc{asfx}")
                            nc.tensor.matmul(pbc, r(ones_row), r(r_row),
                                             start=True, stop=True)
                            # 1/R at full 128-lane width (approx + one NR pass)
                            scratch = work.tile([P, 512], F32, name=f"sc{asfx}",
                                                tag="bc", bufs=1)
                            binv = work.tile([P, 512], F32, name=f"binv{asfx}",
                                             tag="binv", bufs=1)
                            nc.vector.reciprocal_approx_accurate(binv, pbc, scratch)
                            nc.vector.tensor_tensor(
                                hT[:, h, qc * 512:(qc + 1) * 512], pav, binv,
                                mybir.AluOpType.mult)
                        return emit

                    for hh in range(GH):
                        for qc in range(NC2):
                            attn_queue.append(make_attn(g, hh, qc))

                # phase 3 of this batch is deferred: it is emitted after the
                # next batch's first projection group so its matmuls overlap
                # the last attention units
                pending_phase3 = make_phase3(b, hT)

            # tail: drain remaining attention, then the last output projection
            while attn_queue:
                attn_queue.pop(0)()
            if pending_phase3 is not None:
                pending_phase3()

    nc.compile()
    return nc


_NC_CACHE = None


def _get_nc():
    global _NC_CACHE
    if _NC_CACHE is None:
        _NC_CACHE = build_nc()
    return _NC_CACHE


def make_in_maps(x, Wq, bq, Wk, bk, Wv, bv, Wo):
    x = np.asarray(x, dtype=np.float32)
    in_maps = []
    shared = {
        "Wq": np.ascontiguousarray(Wq, dtype=np.float32),
        "Wk": np.ascontiguousarray(Wk, dtype=np.float32),
        "Wv": np.ascontiguousarray(Wv, dtype=np.float32),
        "Wo": np.ascontiguousarray(Wo, dtype=np.float32),
        "bq": np.ascontiguousarray(bq, dtype=np.float32),
        "bk": np.ascontiguousarray(bk, dtype=np.float32),
        "bv": np.ascontiguousarray(bv, dtype=np.float32),
    }
    for c in range(N_CORES):
        xc = x[c * BPC:(c + 1) * BPC]                 # [BPC, N, D]
        xTc = np.ascontiguousarray(xc.transpose(0, 2, 1))  # [BPC, D, N]
        in_maps.append({"xT": xTc, **shared})
    return in_maps


def run(x, Wq, bq, Wk, bk, Wv, bv, Wo, trace=False):
    nc = _get_nc()
    in_maps = make_in_maps(x, Wq, bq, Wk, bk, Wv, bv, Wo)
    res = run_bass_kernel_spmd(nc, in_maps, list(range(N_CORES)), trace=trace)
    out = np.concatenate([res.results[c]["out"] for c in range(N_CORES)], axis=0)
    return out, res


def kernel(x, Wq, bq, Wk, bk, Wv, bv, Wo):
    out, _ = run(x, Wq, bq, Wk, bk, Wv, bv, Wo, trace=False)
    return out


# revision 18
# speedup vs baseline: 1.4868x; 1.0584x over previous
"""Multi-head attention (B=16, N=1024, D=1024, H=8, dh=128) on 8 trn2 cores.

Strategy: data-parallel over batch (2 batches/core), fp32r matmuls.
Per batch on each core:
  phase 1 (per 2-head group g): Q^T_g, K^T_g (head-transposed: dh on
    partitions) and V_g (natural) via fp32r matmuls from x^T (host-side
    pre-transposed) and streamed weight slices.
  phase 2 (per head, per 512-wide q chunk): S^T = K_h^T.T @ Q_h^T (k on
    partitions), E^T = exp(norm*S^T) on ACT, heads^T += V_h.T @ E^T, and
    R = colsum(E^T) via DVE/GpSimd adds + ones-vector matmul; 1/R
    broadcast to all partitions with a K=1 outer-product matmul.
  phase 3: out = (heads^T).T @ Wo in natural layout, scaled by 1/R via
    the broadcast tile, plus bv@Wo row via a K=1 matmul.
"""

import numpy as np

import concourse.bass as bass
import concourse.mybir as mybir
import concourse.tile as tile
from concourse import bacc
from concourse.bass_utils import run_bass_kernel_spmd

N_CORES = 8
B = 16
BPC = B // N_CORES      # batches per core
N = 1024                # sequence length
D = 1024                # model dim
H = 8                   # heads
DH = 128                # head dim
P = 128
DB = D // P             # 8 contraction blocks
GH = 2                  # heads per group
G = H // GH             # 4 groups
GW = GH * DH            # 256: e-width per group
NC2 = N // 512          # 2 n-chunks of 512
NORM = 1.0 / np.sqrt(DH)

F32 = mybir.dt.float32
F32R = mybir.dt.float32r


def r(ap):
    return ap


def build_nc(has_bias=True):
    nc = bacc.Bacc()
    xT = nc.declare_dram_parameter("xT", [BPC, D, N], F32R, isOutput=False)
    Wq = nc.declare_dram_parameter("Wq", [D, D], F32R, isOutput=False)
    Wk = nc.declare_dram_parameter("Wk", [D, D], F32R, isOutput=False)
    Wv = nc.declare_dram_parameter("Wv", [D, D], F32R, isOutput=False)
    Wo = nc.declare_dram_parameter("Wo", [D, D], F32R, isOutput=False)
    bq = nc.declare_dram_parameter("bq", [D], F32, isOutput=False)
    bk = nc.declare_dram_parameter("bk", [D], F32, isOutput=False)
    bv = nc.declare_dram_parameter("bv", [D], F32R, isOutput=False)
    out = nc.declare_dram_parameter("out", [BPC, N, D], F32, isOutput=True)

    ws = [Wq, Wk, Wv]

    with tile.TileContext(nc) as tc:
        with tc.tile_pool(name="big", bufs=1) as big, \
             tc.tile_pool(name="wp", bufs=1) as wp, \
             tc.tile_pool(name="work", bufs=1) as work, \
             tc.tile_pool(name="small", bufs=1) as small, \
             tc.tile_pool(name="ps", bufs=1, space="PSUM") as ps:

            # constants / biases
            bq_col = small.tile([P, DB], F32, name="bq_col")
            bk_col = small.tile([P, DB], F32, name="bk_col")
            bv_col = small.tile([P, DB], F32R, name="bv_col")
            nc.sync.dma_start(out=bq_col, in_=bq.rearrange("(eb p) -> p eb", p=P))
            nc.sync.dma_start(out=bk_col, in_=bk.rearrange("(eb p) -> p eb", p=P))
            nc.sync.dma_start(out=bv_col, in_=bv.rearrange("(eb p) -> p eb", p=P))
            ones_f32 = small.tile([P, 1], F32, name="ones_f32")
            nc.vector.memset(ones_f32, 1.0)
            ones_row_f32 = small.tile([1, P], F32, name="ones_row_f32")
            nc.vector.memset(ones_row_f32, 1.0)
            ones_col = small.tile([P, 1], F32R, name="ones_col")
            nc.vector.tensor_copy(ones_col, ones_f32)
            ones_row = small.tile([1, P], F32R, name="ones_row")
            nc.vector.tensor_copy(ones_row, ones_row_f32)
            c_sb = small.tile([1, NC2, 512], F32R, name="c_sb")

            def emit_proj_unit(b, g, kind, idx, xt, wgt, qTg, kTg, vg):
                """Emit one psum accumulation group of phase 1."""
                gsfx = f"_b{b}_g{g}"
                e0 = g * GW
                if kind in ("q", "k"):
                    dst, wt, bcol = ((qTg, wgt["wq"], bq_col) if kind == "q"
                                     else (kTg, wgt["wk"], bk_col))
                    eb, nch = divmod(idx, NC2)
                    acc = ps.tile([P, 512], F32, tag="pj", bufs=2,
                                  name=f"p{kind}{gsfx}_{eb}_{nch}")
                    for db in range(DB):
                        nc.tensor.matmul(
                            acc,
                            r(wt[:, db, eb * P:(eb + 1) * P]),
                            r(xt[:, db, nch * 512:(nch + 1) * 512]),
                            start=(db == 0), stop=(db == DB - 1))
                    ebg = (e0 // P) + eb
                    if has_bias:
                        nc.vector.tensor_scalar_add(
                            dst[:, eb, nch * 512:(nch + 1) * 512],
                            acc, bcol[:, ebg:ebg + 1])
                    else:
                        nc.vector.tensor_copy(
                            dst[:, eb, nch * 512:(nch + 1) * 512], acc)
                else:  # "v"
                    nb = idx
                    accv = ps.tile([P, 512], F32, tag="pj", bufs=2,
                                   name=f"pv{gsfx}_{nb}")
                    for db in range(DB):
                        nc.tensor.matmul(
                            accv[:, :GW],
                            r(xt[:, db, nb * P:(nb + 1) * P]),
                            r(wgt["wv"][:, db, :]),
                            start=(db == 0), stop=(db == DB - 1))
                    nc.vector.tensor_copy(vg[:, nb, :], accv[:, :GW])

            def make_phase3(b, hT):
                def emit():
                    sfx = f"_b{b}"
                    for oc in range(NC2):
                        wo = wp.tile([P, DB, 512], F32R, name=f"wo{sfx}_{oc}", tag="wo")
                        src = Wo.rearrange("(eb p) o -> p eb o", p=P)
                        nc.sync.dma_start(out=wo[:, 0:DB // 2, :],
                                          in_=src[:, 0:DB // 2, oc * 512:(oc + 1) * 512])
                        nc.sync.dma_start(out=wo[:, DB // 2:, :],
                                          in_=src[:, DB // 2:, oc * 512:(oc + 1) * 512])
                        if b == 0 and has_bias:
                            # c = bv @ Wo for this o-chunk (once; reused for b=1)
                            pc = ps.tile([1, 512], F32, tag="pj", bufs=2, name=f"pc_{oc}")
                            for eb in range(DB):
                                nc.tensor.matmul(pc, r(bv_col[:, eb:eb + 1]),
                                                 r(wo[:, eb, :]),
                                                 start=(eb == 0), stop=(eb == DB - 1))
                            nc.vector.tensor_copy(c_sb[:, oc, :], pc)
                        for nb in range(DB):
                            po = ps.tile([P, 512], F32, tag="pj", bufs=2,
                                         name=f"po{sfx}_{oc}_{nb}")
                            for eb in range(H):
                                nc.tensor.matmul(
                                    po,
                                    r(hT[:, eb, nb * P:(nb + 1) * P]),
                                    r(wo[:, eb, :]),
                                    start=(eb == 0),
                                    stop=(not has_bias and eb == H - 1))
                            if has_bias:
                                nc.tensor.matmul(po, r(ones_row), r(c_sb[:, oc, :]),
                                                 start=False, stop=True)
                            osb = work.tile([P, 512], F32, name=f"o{sfx}_{oc}_{nb}",
                                            tag="osb", bufs=2)
                            nc.scalar.activation(osb, po,
                                                 mybir.ActivationFunctionType.Copy)
                            nc.sync.dma_start(
                                out=out[b, nb * P:(nb + 1) * P, oc * 512:(oc + 1) * 512],
                                in_=osb)
                return emit

            # attention units and the previous batch's output projection are
            # emitted interleaved with later projection units so PE always
            # has ready matmuls during exp/epilogue waits
            attn_queue = []
            pending_phase3 = None

            for b in range(BPC):
                sfx = f"_b{b}"
                # ---- load x^T for this batch: [128, db, n]
                xt = big.tile([P, DB, N], F32R, name=f"xt{sfx}", tag="xt")
                xsrc = xT[b].rearrange("(db p) n -> p db n", p=P)
                for db in range(DB):
                    nc.sync.dma_start(out=xt[:, db, :], in_=xsrc[:, db, :])

                hT = None

                for g in range(G):
                    gsfx = f"{sfx}_g{g}"
                    e0 = g * GW
                    # ---- weight slices for this group: [128, db, GW]
                    wgt = {}
                    for wi, wname in enumerate(("wq", "wk", "wv")):
                        wt = wp.tile([P, DB, GW], F32R, name=f"{wname}{gsfx}", tag="wg", bufs=2)
                        src = ws[wi].rearrange("(db p) e -> p db e", p=P)
                        nc.sync.dma_start(out=wt[:, 0:DB // 2, :],
                                          in_=src[:, 0:DB // 2, e0:e0 + GW])
                        nc.sync.dma_start(out=wt[:, DB // 2:, :],
                                          in_=src[:, DB // 2:, e0:e0 + GW])
                        wgt[wname] = wt

                    qTg = work.tile([P, GH, N], F32R, name=f"qT{gsfx}", tag="qTg", bufs=2)
                    kTg = work.tile([P, GH, N], F32R, name=f"kT{gsfx}", tag="kTg", bufs=2)
                    vg = work.tile([P, DB, GW], F32R, name=f"v{gsfx}", tag="vg", bufs=2)

                    # 16 proj units: 4 Q, 4 K, 8 V; interleave with up to 4
                    # pending attention units (1 attention per 4 proj units)
                    units = ([("q", i) for i in range(GH * NC2)]
                             + [("k", i) for i in range(GH * NC2)]
                             + [("v", i) for i in range(DB)])
                    for ui, (kind, idx) in enumerate(units):
                        emit_proj_unit(b, g, kind, idx, xt, wgt, qTg, kTg, vg)
                        if ui % 4 == 3 and attn_queue:
                            attn_queue.pop(0)()
                    while attn_queue:
                        attn_queue.pop(0)()
                    if pending_phase3 is not None:
                        pending_phase3()
                        pending_phase3 = None
                    if hT is None:
                        hT = big.tile([P, H, N], F32R, name=f"hT{sfx}", tag="hT")

                    # ---- queue attention for the heads of this group
                    def make_attn(g, hh, qc, qTg=qTg, kTg=kTg, vg=vg, hT=hT, b=b):
                        def emit():
                            h = g * GH + hh
                            asfx = f"_b{b}_h{h}_q{qc}"
                            eT = work.tile([P, 4, 1024], F32R, name=f"eT{asfx}",
                                           tag="eT", bufs=2)
                            for j in range(4):
                                # scores for kb=2j, 2j+1 into one 2-bank tile
                                sp = ps.tile([P, 1024], F32, tag="spair", bufs=2,
                                             name=f"sp{asfx}_{j}")
                                for half in range(2):
                                    kb = 2 * j + half
                                    nc.tensor.matmul(
                                        sp[:, half * 512:(half + 1) * 512],
                                        r(kTg[:, hh, kb * P:(kb + 1) * P]),
                                        r(qTg[:, hh, qc * 512:(qc + 1) * 512]),
                                        start=True, stop=True)
                                nc.scalar.activation(
                                    eT[:, j, :], sp,
                                    mybir.ActivationFunctionType.Exp,
                                    scale=float(NORM))

                            # heads^T (unnormalized): [dv(128) x q(512)]
                            pav = ps.tile([P, 512], F32, tag="pav", bufs=1,
                                          name=f"pav{asfx}")
                            for j in range(4):
                                for half in range(2):
                                    kb = 2 * j + half
                                    nc.tensor.matmul(
                                        pav,
                                        r(vg[:, kb, hh * DH:(hh + 1) * DH]),
                                        r(eT[:, j, half * 512:(half + 1) * 512]),
                                        start=(kb == 0), stop=(kb == DB - 1))

                            # R = col-sum of E^T: pairwise adds split DVE/GpSimd
                            add = mybir.AluOpType.add
                            tA = work.tile([P, 1024], F32R, name=f"tA{asfx}", tag="tA", bufs=1)
                            tB = work.tile([P, 1024], F32R, name=f"tB{asfx}", tag="tB", bufs=1)
                            rp = work.tile([P, 512], F32R, name=f"rp{asfx}", tag="rp", bufs=2)
                            nc.vector.tensor_tensor(tA, eT[:, 0, :], eT[:, 1, :], add)
                            nc.vector.tensor_tensor(tB, eT[:, 2, :], eT[:, 3, :], add)
                            nc.vector.tensor_tensor(tA, tA, tB, add)
                            nc.vector.tensor_tensor(rp, tA[:, 0:512], tA[:, 512:1024], add)
                            pr = ps.tile([1, 512], F32, tag="pnorm", bufs=1, name=f"pr{asfx}")
                            nc.tensor.matmul(pr, r(ones_col), r(rp),
                                             start=True, stop=True)
                            # R row to SBUF (rounded to fp32r) then broadcast to
                            # all partitions via a K=1 outer-product matmul
                            r_row = small.tile([1, 512], F32R, name=f"rrow{asfx}",
                                               tag="rinv", bufs=1)
                            nc.vector.tensor_copy(r_row, pr)
                            pbc = ps.tile([P, 512], F32, tag="pnorm", bufs=1, name=f"pbc{asfx}")
                            nc.tensor.matmul(pbc, r(ones_row), r(r_row),
                                             start=True, stop=True)
                            # 1/R at full 128-lane width (approx + one NR pass)
                            scratch = work.tile([P, 512], F32, name=f"sc{asfx}",
                                                tag="bc", bufs=1)
                            binv = work.tile([P, 512], F32, name=f"binv{asfx}",
                                             tag="binv", bufs=1)
                            nc.vector.reciprocal_approx_accurate(binv, pbc, scratch)
                            nc.vector.tensor_tensor(
                                hT[:, h, qc * 512:(qc + 1) * 512], pav, binv,
                                mybir.AluOpType.mult)
                        return emit

                    for hh in range(GH):
                        for qc in range(NC2):
                            attn_queue.append(make_attn(g, hh, qc))

                # phase 3 of this batch is deferred: it is emitted after the
                # next batch's first projection group so its matmuls overlap
                # the last attention units
                pending_phase3 = make_phase3(b, hT)

            # tail: drain remaining attention, then the last output projection
            while attn_queue:
                attn_queue.pop(0)()
            if pending_phase3 is not None:
                pending_phase3()

    nc.compile()
    return nc


_NC_CACHE = {}


def _get_nc(has_bias):
    if has_bias not in _NC_CACHE:
        _NC_CACHE[has_bias] = build_nc(has_bias)
    return _NC_CACHE[has_bias]


def make_in_maps(x, Wq, bq, Wk, bk, Wv, bv, Wo):
    x = np.asarray(x, dtype=np.float32)
    in_maps = []
    shared = {
        "Wq": np.ascontiguousarray(Wq, dtype=np.float32),
        "Wk": np.ascontiguousarray(Wk, dtype=np.float32),
        "Wv": np.ascontiguousarray(Wv, dtype=np.float32),
        "Wo": np.ascontiguousarray(Wo, dtype=np.float32),
        "bq": np.ascontiguousarray(bq, dtype=np.float32),
        "bk": np.ascontiguousarray(bk, dtype=np.float32),
        "bv": np.ascontiguousarray(bv, dtype=np.float32),
    }
    for c in range(N_CORES):
        xc = x[c * BPC:(c + 1) * BPC]                 # [BPC, N, D]
        xTc = np.ascontiguousarray(xc.transpose(0, 2, 1))  # [BPC, D, N]
        in_maps.append({"xT": xTc, **shared})
    return in_maps


def run(x, Wq, bq, Wk, bk, Wv, bv, Wo, trace=False):
    has_bias = bool(np.any(np.asarray(bq)) or np.any(np.asarray(bk))
                    or np.any(np.asarray(bv)))
    nc = _get_nc(has_bias)
    in_maps = make_in_maps(x, Wq, bq, Wk, bk, Wv, bv, Wo)
    res = run_bass_kernel_spmd(nc, in_maps, list(range(N_CORES)), trace=trace)
    out = np.concatenate([res.results[c]["out"] for c in range(N_CORES)], axis=0)
    return out, res


def kernel(x, Wq, bq, Wk, bk, Wv, bv, Wo):
    out, _ = run(x, Wq, bq, Wk, bk, Wv, bv, Wo, trace=False)
    return out


# revision 19
# speedup vs baseline: 1.5795x; 1.0623x over previous
"""Multi-head attention (B=16, N=1024, D=1024, H=8, dh=128) on 8 trn2 cores.

Strategy: data-parallel over batch (2 batches/core), fp32r matmuls.
Per batch on each core:
  phase 1 (per 2-head group g): Q^T_g, K^T_g (head-transposed: dh on
    partitions) and V_g (natural) via fp32r matmuls from x^T (host-side
    pre-transposed) and streamed weight slices.
  phase 2 (per head, per 512-wide q chunk): S^T = K_h^T.T @ Q_h^T (k on
    partitions), E^T = exp(norm*S^T) on ACT, heads^T += V_h.T @ E^T, and
    R = colsum(E^T) via DVE/GpSimd adds + ones-vector matmul; 1/R
    broadcast to all partitions with a K=1 outer-product matmul.
  phase 3: out = (heads^T).T @ Wo in natural layout, scaled by 1/R via
    the broadcast tile, plus bv@Wo row via a K=1 matmul.
"""

import numpy as np

import concourse.bass as bass
import concourse.mybir as mybir
import concourse.tile as tile
from concourse import bacc
from concourse.bass_utils import run_bass_kernel_spmd

N_CORES = 8
B = 16
BPC = B // N_CORES      # batches per core
N = 1024                # sequence length
D = 1024                # model dim
H = 8                   # heads
DH = 128                # head dim
P = 128
DB = D // P             # 8 contraction blocks
GH = 2                  # heads per group
G = H // GH             # 4 groups
GW = GH * DH            # 256: e-width per group
NC2 = N // 512          # 2 n-chunks of 512
NORM = 1.0 / np.sqrt(DH)

F32 = mybir.dt.float32
F32R = mybir.dt.float32r


def r(ap):
    return ap


def build_nc(has_bias=True):
    nc = bacc.Bacc()
    xT = nc.declare_dram_parameter("xT", [BPC, D, N], F32R, isOutput=False)
    Wq = nc.declare_dram_parameter("Wq", [D, D], F32R, isOutput=False)
    Wk = nc.declare_dram_parameter("Wk", [D, D], F32R, isOutput=False)
    Wv = nc.declare_dram_parameter("Wv", [D, D], F32R, isOutput=False)
    Wo = nc.declare_dram_parameter("Wo", [D, D], F32R, isOutput=False)
    bq = nc.declare_dram_parameter("bq", [D], F32, isOutput=False)
    bk = nc.declare_dram_parameter("bk", [D], F32, isOutput=False)
    bv = nc.declare_dram_parameter("bv", [D], F32R, isOutput=False)
    out = nc.declare_dram_parameter("out", [BPC, N, D], F32, isOutput=True)

    ws = [Wq, Wk, Wv]

    with tile.TileContext(nc) as tc:
        with tc.tile_pool(name="big", bufs=1) as big, \
             tc.tile_pool(name="wp", bufs=1) as wp, \
             tc.tile_pool(name="work", bufs=1) as work, \
             tc.tile_pool(name="small", bufs=1) as small, \
             tc.tile_pool(name="ps", bufs=1, space="PSUM") as ps:

            # constants / biases
            bq_col = small.tile([P, DB], F32, name="bq_col")
            bk_col = small.tile([P, DB], F32, name="bk_col")
            bv_col = small.tile([P, DB], F32R, name="bv_col")
            nc.sync.dma_start(out=bq_col, in_=bq.rearrange("(eb p) -> p eb", p=P))
            nc.sync.dma_start(out=bk_col, in_=bk.rearrange("(eb p) -> p eb", p=P))
            nc.sync.dma_start(out=bv_col, in_=bv.rearrange("(eb p) -> p eb", p=P))
            ones_f32 = small.tile([P, 1], F32, name="ones_f32")
            nc.vector.memset(ones_f32, 1.0)
            ones_row_f32 = small.tile([1, P], F32, name="ones_row_f32")
            nc.vector.memset(ones_row_f32, 1.0)
            ones_col = small.tile([P, 1], F32R, name="ones_col")
            nc.vector.tensor_copy(ones_col, ones_f32)
            ones_row = small.tile([1, P], F32R, name="ones_row")
            nc.vector.tensor_copy(ones_row, ones_row_f32)
            ones128_f32 = small.tile([P, P], F32, name="ones128_f32")
            nc.vector.memset(ones128_f32, 1.0)
            ones128 = small.tile([P, P], F32R, name="ones128")
            nc.vector.tensor_copy(ones128, ones128_f32)
            c_sb = small.tile([1, NC2, 512], F32R, name="c_sb")

            def emit_proj_unit(b, g, kind, idx, xt, wgt, qTg, kTg, vg):
                """Emit one psum accumulation group of phase 1."""
                gsfx = f"_b{b}_g{g}"
                e0 = g * GW
                if kind in ("q", "k"):
                    dst, wt, bcol = ((qTg, wgt["wq"], bq_col) if kind == "q"
                                     else (kTg, wgt["wk"], bk_col))
                    eb, nch = divmod(idx, NC2)
                    acc = ps.tile([P, 512], F32, tag="pj", bufs=2,
                                  name=f"p{kind}{gsfx}_{eb}_{nch}")
                    for db in range(DB):
                        nc.tensor.matmul(
                            acc,
                            r(wt[:, db, eb * P:(eb + 1) * P]),
                            r(xt[:, db, nch * 512:(nch + 1) * 512]),
                            start=(db == 0), stop=(db == DB - 1))
                    ebg = (e0 // P) + eb
                    if has_bias:
                        nc.vector.tensor_scalar_add(
                            dst[:, eb, nch * 512:(nch + 1) * 512],
                            acc, bcol[:, ebg:ebg + 1])
                    else:
                        nc.vector.tensor_copy(
                            dst[:, eb, nch * 512:(nch + 1) * 512], acc)
                else:  # "v"
                    nb = idx
                    accv = ps.tile([P, 512], F32, tag="pj", bufs=2,
                                   name=f"pv{gsfx}_{nb}")
                    for db in range(DB):
                        nc.tensor.matmul(
                            accv[:, :GW],
                            r(xt[:, db, nb * P:(nb + 1) * P]),
                            r(wgt["wv"][:, db, :]),
                            start=(db == 0), stop=(db == DB - 1))
                    nc.vector.tensor_copy(vg[:, nb, :], accv[:, :GW])

            def make_phase3(b, hT):
                def emit():
                    sfx = f"_b{b}"
                    for oc in range(NC2):
                        wo = wp.tile([P, DB, 512], F32R, name=f"wo{sfx}_{oc}", tag="wo")
                        src = Wo.rearrange("(eb p) o -> p eb o", p=P)
                        nc.sync.dma_start(out=wo[:, 0:DB // 2, :],
                                          in_=src[:, 0:DB // 2, oc * 512:(oc + 1) * 512])
                        nc.sync.dma_start(out=wo[:, DB // 2:, :],
                                          in_=src[:, DB // 2:, oc * 512:(oc + 1) * 512])
                        if b == 0 and has_bias:
                            # c = bv @ Wo for this o-chunk (once; reused for b=1)
                            pc = ps.tile([1, 512], F32, tag="pj", bufs=2, name=f"pc_{oc}")
                            for eb in range(DB):
                                nc.tensor.matmul(pc, r(bv_col[:, eb:eb + 1]),
                                                 r(wo[:, eb, :]),
                                                 start=(eb == 0), stop=(eb == DB - 1))
                            nc.vector.tensor_copy(c_sb[:, oc, :], pc)
                        for nb in range(DB):
                            po = ps.tile([P, 512], F32, tag="pj", bufs=2,
                                         name=f"po{sfx}_{oc}_{nb}")
                            for eb in range(H):
                                nc.tensor.matmul(
                                    po,
                                    r(hT[:, eb, nb * P:(nb + 1) * P]),
                                    r(wo[:, eb, :]),
                                    start=(eb == 0),
                                    stop=(not has_bias and eb == H - 1))
                            if has_bias:
                                nc.tensor.matmul(po, r(ones_row), r(c_sb[:, oc, :]),
                                                 start=False, stop=True)
                            osb = work.tile([P, 512], F32, name=f"o{sfx}_{oc}_{nb}",
                                            tag="osb", bufs=2)
                            nc.scalar.activation(osb, po,
                                                 mybir.ActivationFunctionType.Copy)
                            nc.sync.dma_start(
                                out=out[b, nb * P:(nb + 1) * P, oc * 512:(oc + 1) * 512],
                                in_=osb)
                return emit

            # attention units and the previous batch's output projection are
            # emitted interleaved with later projection units so PE always
            # has ready matmuls during exp/epilogue waits
            attn_queue = []
            pending_phase3 = None

            for b in range(BPC):
                sfx = f"_b{b}"
                # ---- load x^T for this batch: [128, db, n]
                xt = big.tile([P, DB, N], F32R, name=f"xt{sfx}", tag="xt")
                xsrc = xT[b].rearrange("(db p) n -> p db n", p=P)
                for db in range(DB):
                    nc.sync.dma_start(out=xt[:, db, :], in_=xsrc[:, db, :])

                hT = None

                for g in range(G):
                    gsfx = f"{sfx}_g{g}"
                    e0 = g * GW
                    # ---- weight slices for this group: [128, db, GW]
                    wgt = {}
                    for wi, wname in enumerate(("wq", "wk", "wv")):
                        wt = wp.tile([P, DB, GW], F32R, name=f"{wname}{gsfx}", tag="wg", bufs=2)
                        src = ws[wi].rearrange("(db p) e -> p db e", p=P)
                        nc.sync.dma_start(out=wt[:, 0:DB // 2, :],
                                          in_=src[:, 0:DB // 2, e0:e0 + GW])
                        nc.sync.dma_start(out=wt[:, DB // 2:, :],
                                          in_=src[:, DB // 2:, e0:e0 + GW])
                        wgt[wname] = wt

                    qTg = work.tile([P, GH, N], F32R, name=f"qT{gsfx}", tag="qTg", bufs=2)
                    kTg = work.tile([P, GH, N], F32R, name=f"kT{gsfx}", tag="kTg", bufs=2)
                    vg = work.tile([P, DB, GW], F32R, name=f"v{gsfx}", tag="vg", bufs=2)

                    # 16 proj units: 4 Q, 4 K, 8 V; interleave with up to 4
                    # pending attention units (1 attention per 4 proj units)
                    units = ([("q", i) for i in range(GH * NC2)]
                             + [("k", i) for i in range(GH * NC2)]
                             + [("v", i) for i in range(DB)])
                    for ui, (kind, idx) in enumerate(units):
                        emit_proj_unit(b, g, kind, idx, xt, wgt, qTg, kTg, vg)
                        if ui % 4 == 3 and attn_queue:
                            attn_queue.pop(0)()
                    while attn_queue:
                        attn_queue.pop(0)()
                    if pending_phase3 is not None:
                        pending_phase3()
                        pending_phase3 = None
                    if hT is None:
                        hT = big.tile([P, H, N], F32R, name=f"hT{sfx}", tag="hT")

                    # ---- queue attention for the heads of this group
                    def make_attn(g, hh, qc, qTg=qTg, kTg=kTg, vg=vg, hT=hT, b=b):
                        def emit():
                            h = g * GH + hh
                            asfx = f"_b{b}_h{h}_q{qc}"
                            eT = work.tile([P, 4, 1024], F32R, name=f"eT{asfx}",
                                           tag="eT", bufs=2)
                            for j in range(4):
                                # scores for kb=2j, 2j+1 into one 2-bank tile
                                sp = ps.tile([P, 1024], F32, tag="spair", bufs=2,
                                             name=f"sp{asfx}_{j}")
                                for half in range(2):
                                    kb = 2 * j + half
                                    nc.tensor.matmul(
                                        sp[:, half * 512:(half + 1) * 512],
                                        r(kTg[:, hh, kb * P:(kb + 1) * P]),
                                        r(qTg[:, hh, qc * 512:(qc + 1) * 512]),
                                        start=True, stop=True)
                                nc.scalar.activation(
                                    eT[:, j, :], sp,
                                    mybir.ActivationFunctionType.Exp,
                                    scale=float(NORM))

                            # heads^T (unnormalized): [dv(128) x q(512)]
                            pav = ps.tile([P, 512], F32, tag="pav", bufs=1,
                                          name=f"pav{asfx}")
                            for j in range(4):
                                for half in range(2):
                                    kb = 2 * j + half
                                    nc.tensor.matmul(
                                        pav,
                                        r(vg[:, kb, hh * DH:(hh + 1) * DH]),
                                        r(eT[:, j, half * 512:(half + 1) * 512]),
                                        start=(kb == 0), stop=(kb == DB - 1))

                            # R = col-sum of E^T: pairwise adds split DVE/GpSimd
                            add = mybir.AluOpType.add
                            tA = work.tile([P, 1024], F32R, name=f"tA{asfx}", tag="tA", bufs=1)
                            tB = work.tile([P, 1024], F32R, name=f"tB{asfx}", tag="tB", bufs=1)
                            rp = work.tile([P, 512], F32R, name=f"rp{asfx}", tag="rp", bufs=2)
                            nc.vector.tensor_tensor(tA, eT[:, 0, :], eT[:, 1, :], add)
                            nc.vector.tensor_tensor(tB, eT[:, 2, :], eT[:, 3, :], add)
                            nc.vector.tensor_tensor(tA, tA, tB, add)
                            nc.vector.tensor_tensor(rp, tA[:, 0:512], tA[:, 512:1024], add)
                            # colsum of rp, broadcast to all partitions, in
                            # one matmul: every row of ones128.T @ rp is R
                            pbc = ps.tile([P, 512], F32, tag="pnorm", bufs=1, name=f"pbc{asfx}")
                            nc.tensor.matmul(pbc, r(ones128), r(rp),
                                             start=True, stop=True)
                            # 1/R at full 128-lane width (approx + one NR pass)
                            scratch = work.tile([P, 512], F32, name=f"sc{asfx}",
                                                tag="bc", bufs=1)
                            binv = work.tile([P, 512], F32, name=f"binv{asfx}",
                                             tag="binv", bufs=1)
                            nc.vector.reciprocal_approx_accurate(binv, pbc, scratch)
                            nc.vector.tensor_tensor(
                                hT[:, h, qc * 512:(qc + 1) * 512], pav, binv,
                                mybir.AluOpType.mult)
                        return emit

                    for hh in range(GH):
                        for qc in range(NC2):
                            attn_queue.append(make_attn(g, hh, qc))

                # phase 3 of this batch is deferred: it is emitted after the
                # next batch's first projection group so its matmuls overlap
                # the last attention units
                pending_phase3 = make_phase3(b, hT)

            # tail: drain remaining attention, then the last output projection
            while attn_queue:
                attn_queue.pop(0)()
            if pending_phase3 is not None:
                pending_phase3()

    nc.compile()
    return nc


_NC_CACHE = {}


def _get_nc(has_bias):
    if has_bias not in _NC_CACHE:
        _NC_CACHE[has_bias] = build_nc(has_bias)
    return _NC_CACHE[has_bias]


def make_in_maps(x, Wq, bq, Wk, bk, Wv, bv, Wo):
    x = np.asarray(x, dtype=np.float32)
    in_maps = []
    shared = {
        "Wq": np.ascontiguousarray(Wq, dtype=np.float32),
        "Wk": np.ascontiguousarray(Wk, dtype=np.float32),
        "Wv": np.ascontiguousarray(Wv, dtype=np.float32),
        "Wo": np.ascontiguousarray(Wo, dtype=np.float32),
        "bq": np.ascontiguousarray(bq, dtype=np.float32),
        "bk": np.ascontiguousarray(bk, dtype=np.float32),
        "bv": np.ascontiguousarray(bv, dtype=np.float32),
    }
    for c in range(N_CORES):
        xc = x[c * BPC:(c + 1) * BPC]                 # [BPC, N, D]
        xTc = np.ascontiguousarray(xc.transpose(0, 2, 1))  # [BPC, D, N]
        in_maps.append({"xT": xTc, **shared})
    return in_maps


def run(x, Wq, bq, Wk, bk, Wv, bv, Wo, trace=False):
    has_bias = bool(np.any(np.asarray(bq)) or np.any(np.asarray(bk))
                    or np.any(np.asarray(bv)))
    nc = _get_nc(has_bias)
    in_maps = make_in_maps(x, Wq, bq, Wk, bk, Wv, bv, Wo)
    res = run_bass_kernel_spmd(nc, in_maps, list(range(N_CORES)), trace=trace)
    out = np.concatenate([res.results[c]["out"] for c in range(N_CORES)], axis=0)
    return out, res


def kernel(x, Wq, bq, Wk, bk, Wv, bv, Wo):
    out, _ = run(x, Wq, bq, Wk, bk, Wv, bv, Wo, trace=False)
    return out


# revision 21
# speedup vs baseline: 1.5961x; 1.0106x over previous
"""Multi-head attention (B=16, N=1024, D=1024, H=8, dh=128) on 8 trn2 cores.

Strategy: data-parallel over batch (2 batches/core), fp32r matmuls.
Per batch on each core:
  phase 1 (per 2-head group g): Q^T_g, K^T_g (head-transposed: dh on
    partitions) and V_g (natural) via fp32r matmuls from x^T (host-side
    pre-transposed) and streamed weight slices.
  phase 2 (per head, per 512-wide q chunk): S^T = K_h^T.T @ Q_h^T (k on
    partitions), E^T = exp(norm*S^T) on ACT, heads^T += V_h.T @ E^T, and
    R = colsum(E^T) via DVE pairwise adds, then one all-ones 128x128
    matmul that yields R already broadcast to every partition; 1/R via
    a fast 128-lane reciprocal, applied while copying heads^T to SBUF.
  phase 3: out = (heads_norm) @ Wo in natural layout (+ bv@Wo row via a
    K=1 matmul when biases are nonzero).
"""

import numpy as np

import concourse.bass as bass
import concourse.mybir as mybir
import concourse.tile as tile
from concourse import bacc
from concourse.bass_utils import run_bass_kernel_spmd

N_CORES = 8
B = 16
BPC = B // N_CORES      # batches per core
N = 1024                # sequence length
D = 1024                # model dim
H = 8                   # heads
DH = 128                # head dim
P = 128
DB = D // P             # 8 contraction blocks
GH = 2                  # heads per group
G = H // GH             # 4 groups
GW = GH * DH            # 256: e-width per group
NC2 = N // 512          # 2 n-chunks of 512
NORM = 1.0 / np.sqrt(DH)

F32 = mybir.dt.float32
F32R = mybir.dt.float32r


def r(ap):
    return ap


def build_nc(has_bias=True):
    nc = bacc.Bacc()
    xT = nc.declare_dram_parameter("xT", [BPC, D, N], F32R, isOutput=False)
    Wq = nc.declare_dram_parameter("Wq", [D, D], F32R, isOutput=False)
    Wk = nc.declare_dram_parameter("Wk", [D, D], F32R, isOutput=False)
    Wv = nc.declare_dram_parameter("Wv", [D, D], F32R, isOutput=False)
    Wo = nc.declare_dram_parameter("Wo", [D, D], F32R, isOutput=False)
    bq = nc.declare_dram_parameter("bq", [D], F32, isOutput=False)
    bk = nc.declare_dram_parameter("bk", [D], F32, isOutput=False)
    bv = nc.declare_dram_parameter("bv", [D], F32R, isOutput=False)
    out = nc.declare_dram_parameter("out", [BPC, N, D], F32, isOutput=True)

    ws = [Wq, Wk, Wv]

    with tile.TileContext(nc) as tc:
        with tc.tile_pool(name="big", bufs=1) as big, \
             tc.tile_pool(name="wp", bufs=1) as wp, \
             tc.tile_pool(name="work", bufs=1) as work, \
             tc.tile_pool(name="small", bufs=1) as small, \
             tc.tile_pool(name="ps", bufs=1, space="PSUM") as ps:

            # constants / biases
            bq_col = small.tile([P, DB], F32, name="bq_col")
            bk_col = small.tile([P, DB], F32, name="bk_col")
            bv_col = small.tile([P, DB], F32R, name="bv_col")
            nc.sync.dma_start(out=bq_col, in_=bq.rearrange("(eb p) -> p eb", p=P))
            nc.sync.dma_start(out=bk_col, in_=bk.rearrange("(eb p) -> p eb", p=P))
            nc.sync.dma_start(out=bv_col, in_=bv.rearrange("(eb p) -> p eb", p=P))
            ones_f32 = small.tile([P, 1], F32, name="ones_f32")
            nc.vector.memset(ones_f32, 1.0)
            ones_row_f32 = small.tile([1, P], F32, name="ones_row_f32")
            nc.vector.memset(ones_row_f32, 1.0)
            ones_col = small.tile([P, 1], F32R, name="ones_col")
            nc.vector.tensor_copy(ones_col, ones_f32)
            ones_row = small.tile([1, P], F32R, name="ones_row")
            nc.vector.tensor_copy(ones_row, ones_row_f32)
            ones128_f32 = small.tile([P, P], F32, name="ones128_f32")
            nc.vector.memset(ones128_f32, 1.0)
            ones128 = small.tile([P, P], F32R, name="ones128")
            nc.vector.tensor_copy(ones128, ones128_f32)
            c_sb = small.tile([1, NC2, 512], F32R, name="c_sb")

            def emit_proj_unit(b, g, kind, idx, xt, wgt, qTg, kTg, vg):
                """Emit one psum accumulation group of phase 1."""
                gsfx = f"_b{b}_g{g}"
                e0 = g * GW
                if kind in ("q", "k"):
                    dst, wt, bcol = ((qTg, wgt["wq"], bq_col) if kind == "q"
                                     else (kTg, wgt["wk"], bk_col))
                    eb, nch = divmod(idx, NC2)
                    acc = ps.tile([P, 512], F32, tag="pj", bufs=2,
                                  name=f"p{kind}{gsfx}_{eb}_{nch}")
                    for db in range(DB):
                        nc.tensor.matmul(
                            acc,
                            r(wt[:, db, eb * P:(eb + 1) * P]),
                            r(xt[:, db, nch * 512:(nch + 1) * 512]),
                            start=(db == 0), stop=(db == DB - 1))
                    ebg = (e0 // P) + eb
                    if has_bias:
                        nc.vector.tensor_scalar_add(
                            dst[:, eb, nch * 512:(nch + 1) * 512],
                            acc, bcol[:, ebg:ebg + 1])
                    else:
                        nc.vector.tensor_copy(
                            dst[:, eb, nch * 512:(nch + 1) * 512], acc)
                else:  # "v"
                    nb = idx
                    accv = ps.tile([P, 512], F32, tag="pj", bufs=2,
                                   name=f"pv{gsfx}_{nb}")
                    for db in range(DB):
                        nc.tensor.matmul(
                            accv[:, :GW],
                            r(xt[:, db, nb * P:(nb + 1) * P]),
                            r(wgt["wv"][:, db, :]),
                            start=(db == 0), stop=(db == DB - 1))
                    nc.vector.tensor_copy(vg[:, nb, :], accv[:, :GW])

            def make_phase3(b, hT):
                def emit():
                    sfx = f"_b{b}"
                    for oc in range(NC2):
                        wo = wp.tile([P, DB, 512], F32R, name=f"wo{sfx}_{oc}", tag="wo")
                        src = Wo.rearrange("(eb p) o -> p eb o", p=P)
                        nc.sync.dma_start(out=wo[:, 0:DB // 2, :],
                                          in_=src[:, 0:DB // 2, oc * 512:(oc + 1) * 512])
                        nc.sync.dma_start(out=wo[:, DB // 2:, :],
                                          in_=src[:, DB // 2:, oc * 512:(oc + 1) * 512])
                        if b == 0 and has_bias:
                            # c = bv @ Wo for this o-chunk (once; reused for b=1)
                            pc = ps.tile([1, 512], F32, tag="pj", bufs=2, name=f"pc_{oc}")
                            for eb in range(DB):
                                nc.tensor.matmul(pc, r(bv_col[:, eb:eb + 1]),
                                                 r(wo[:, eb, :]),
                                                 start=(eb == 0), stop=(eb == DB - 1))
                            nc.vector.tensor_copy(c_sb[:, oc, :], pc)
                        for nb in range(DB):
                            po = ps.tile([P, 512], F32, tag="pj", bufs=2,
                                         name=f"po{sfx}_{oc}_{nb}")
                            for eb in range(H):
                                nc.tensor.matmul(
                                    po,
                                    r(hT[:, eb, nb * P:(nb + 1) * P]),
                                    r(wo[:, eb, :]),
                                    start=(eb == 0),
                                    stop=(not has_bias and eb == H - 1))
                            if has_bias:
                                nc.tensor.matmul(po, r(ones_row), r(c_sb[:, oc, :]),
                                                 start=False, stop=True)
                            osb = work.tile([P, 512], F32, name=f"o{sfx}_{oc}_{nb}",
                                            tag="osb", bufs=2)
                            nc.scalar.activation(osb, po,
                                                 mybir.ActivationFunctionType.Copy)
                            nc.sync.dma_start(
                                out=out[b, nb * P:(nb + 1) * P, oc * 512:(oc + 1) * 512],
                                in_=osb)
                return emit

            # attention units and the previous batch's output projection are
            # emitted interleaved with later projection units so PE always
            # has ready matmuls during exp/epilogue waits
            attn_queue = []
            pending_phase3 = None

            for b in range(BPC):
                sfx = f"_b{b}"
                # ---- load x^T for this batch: [128, db, n]
                xt = big.tile([P, DB, N], F32R, name=f"xt{sfx}", tag="xt")
                xsrc = xT[b].rearrange("(db p) n -> p db n", p=P)
                for db in range(DB):
                    nc.sync.dma_start(out=xt[:, db, :], in_=xsrc[:, db, :])

                hT = None

                for g in range(G):
                    gsfx = f"{sfx}_g{g}"
                    e0 = g * GW
                    # ---- weight slices for this group: [128, db, GW]
                    wgt = {}
                    for wi, wname in enumerate(("wq", "wk", "wv")):
                        wt = wp.tile([P, DB, GW], F32R, name=f"{wname}{gsfx}", tag="wg", bufs=2)
                        src = ws[wi].rearrange("(db p) e -> p db e", p=P)
                        nc.sync.dma_start(out=wt[:, 0:DB // 2, :],
                                          in_=src[:, 0:DB // 2, e0:e0 + GW])
                        nc.sync.dma_start(out=wt[:, DB // 2:, :],
                                          in_=src[:, DB // 2:, e0:e0 + GW])
                        wgt[wname] = wt

                    qTg = work.tile([P, GH, N], F32R, name=f"qT{gsfx}", tag="qTg", bufs=2)
                    kTg = work.tile([P, GH, N], F32R, name=f"kT{gsfx}", tag="kTg", bufs=2)
                    vg = work.tile([P, DB, GW], F32R, name=f"v{gsfx}", tag="vg", bufs=2)

                    # 16 proj units: 4 Q, 4 K, 8 V; interleave with up to 4
                    # pending attention units (1 attention per 4 proj units)
                    units = ([("q", i) for i in range(GH * NC2)]
                             + [("k", i) for i in range(GH * NC2)]
                             + [("v", i) for i in range(DB)])
                    for ui, (kind, idx) in enumerate(units):
                        emit_proj_unit(b, g, kind, idx, xt, wgt, qTg, kTg, vg)
                        if ui % 4 == 1 and attn_queue:
                            attn_queue.pop(0)()
                    while attn_queue:
                        attn_queue.pop(0)()
                    if pending_phase3 is not None:
                        pending_phase3()
                        pending_phase3 = None
                    if hT is None:
                        hT = big.tile([P, H, N], F32R, name=f"hT{sfx}", tag="hT")

                    # ---- queue attention for the heads of this group
                    def make_attn(g, hh, qc, qTg=qTg, kTg=kTg, vg=vg, hT=hT, b=b):
                        def emit():
                            h = g * GH + hh
                            asfx = f"_b{b}_h{h}_q{qc}"
                            eT = work.tile([P, 4, 1024], F32R, name=f"eT{asfx}",
                                           tag="eT", bufs=2)
                            for j in range(4):
                                # scores for kb=2j, 2j+1 into one 2-bank tile
                                sp = ps.tile([P, 1024], F32, tag="spair", bufs=2,
                                             name=f"sp{asfx}_{j}")
                                for half in range(2):
                                    kb = 2 * j + half
                                    nc.tensor.matmul(
                                        sp[:, half * 512:(half + 1) * 512],
                                        r(kTg[:, hh, kb * P:(kb + 1) * P]),
                                        r(qTg[:, hh, qc * 512:(qc + 1) * 512]),
                                        start=True, stop=True)
                                nc.scalar.activation(
                                    eT[:, j, :], sp,
                                    mybir.ActivationFunctionType.Exp,
                                    scale=float(NORM))

                            # heads^T (unnormalized): [dv(128) x q(512)]
                            pav = ps.tile([P, 512], F32, tag="pav", bufs=1,
                                          name=f"pav{asfx}")
                            for j in range(4):
                                for half in range(2):
                                    kb = 2 * j + half
                                    nc.tensor.matmul(
                                        pav,
                                        r(vg[:, kb, hh * DH:(hh + 1) * DH]),
                                        r(eT[:, j, half * 512:(half + 1) * 512]),
                                        start=(kb == 0), stop=(kb == DB - 1))

                            # R = col-sum of E^T: pairwise adds split DVE/GpSimd
                            add = mybir.AluOpType.add
                            tA = work.tile([P, 1024], F32R, name=f"tA{asfx}", tag="tA", bufs=1)
                            tB = work.tile([P, 1024], F32R, name=f"tB{asfx}", tag="tB", bufs=1)
                            rp = work.tile([P, 512], F32R, name=f"rp{asfx}", tag="rp", bufs=2)
                            nc.vector.tensor_tensor(tA, eT[:, 0, :], eT[:, 1, :], add)
                            nc.vector.tensor_tensor(tB, eT[:, 2, :], eT[:, 3, :], add)
                            nc.vector.tensor_tensor(tA, tA, tB, add)
                            nc.vector.tensor_tensor(rp, tA[:, 0:512], tA[:, 512:1024], add)
                            # colsum of rp, broadcast to all partitions, in
                            # one matmul: every row of ones128.T @ rp is R
                            pbc = ps.tile([P, 512], F32, tag="pnorm", bufs=1, name=f"pbc{asfx}")
                            nc.tensor.matmul(pbc, r(ones128), r(rp),
                                             start=True, stop=True)
                            # 1/R at full 128-lane width (approx + one NR pass)
                            scratch = work.tile([P, 512], F32, name=f"sc{asfx}",
                                                tag="bc", bufs=1)
                            binv = work.tile([P, 512], F32, name=f"binv{asfx}",
                                             tag="binv", bufs=1)
                            nc.vector.reciprocal_approx_accurate(binv, pbc, scratch)
                            nc.vector.tensor_tensor(
                                hT[:, h, qc * 512:(qc + 1) * 512], pav, binv,
                                mybir.AluOpType.mult)
                        return emit

                    for hh in range(GH):
                        for qc in range(NC2):
                            attn_queue.append(make_attn(g, hh, qc))

                # phase 3 of this batch is deferred: it is emitted after the
                # next batch's first projection group so its matmuls overlap
                # the last attention units
                pending_phase3 = make_phase3(b, hT)

            # tail: drain remaining attention, then the last output projection
            while attn_queue:
                attn_queue.pop(0)()
            if pending_phase3 is not None:
                pending_phase3()

    nc.compile()
    return nc


_NC_CACHE = {}


def _get_nc(has_bias):
    if has_bias not in _NC_CACHE:
        _NC_CACHE[has_bias] = build_nc(has_bias)
    return _NC_CACHE[has_bias]


def make_in_maps(x, Wq, bq, Wk, bk, Wv, bv, Wo):
    x = np.asarray(x, dtype=np.float32)
    in_maps = []
    shared = {
        "Wq": np.ascontiguousarray(Wq, dtype=np.float32),
        "Wk": np.ascontiguousarray(Wk, dtype=np.float32),
        "Wv": np.ascontiguousarray(Wv, dtype=np.float32),
        "Wo": np.ascontiguousarray(Wo, dtype=np.float32),
        "bq": np.ascontiguousarray(bq, dtype=np.float32),
        "bk": np.ascontiguousarray(bk, dtype=np.float32),
        "bv": np.ascontiguousarray(bv, dtype=np.float32),
    }
    for c in range(N_CORES):
        xc = x[c * BPC:(c + 1) * BPC]                 # [BPC, N, D]
        xTc = np.ascontiguousarray(xc.transpose(0, 2, 1))  # [BPC, D, N]
        in_maps.append({"xT": xTc, **shared})
    return in_maps


def run(x, Wq, bq, Wk, bk, Wv, bv, Wo, trace=False):
    has_bias = bool(np.any(np.asarray(bq)) or np.any(np.asarray(bk))
                    or np.any(np.asarray(bv)))
    nc = _get_nc(has_bias)
    in_maps = make_in_maps(x, Wq, bq, Wk, bk, Wv, bv, Wo)
    res = run_bass_kernel_spmd(nc, in_maps, list(range(N_CORES)), trace=trace)
    out = np.concatenate([res.results[c]["out"] for c in range(N_CORES)], axis=0)
    return out, res


def kernel(x, Wq, bq, Wk, bk, Wv, bv, Wo):
    out, _ = run(x, Wq, bq, Wk, bk, Wv, bv, Wo, trace=False)
    return out
